# revision 1
# baseline (speedup 1.0000x reference)
"""HGNN (2-layer hetero GraphSAGE + 8 heads) on 8 trn2 NeuronCores.

Sharding: dst-node interleaved (core = v % 8, local = v // 8). Each layer is
one SPMD NEFF launch; the host performs the inter-layer halo exchange by
concatenating per-core outputs into fresh gather tables (indices are
pre-translated into the concatenated layout).

Device-side per layer, per core:
  - For each 512-dst-column PSUM group, edges (sorted by dst) are cut into
    128-edge windows on a column grid that is uniform across cores
    (min-over-cores advance), so a single program serves all 8 cores.
  - Per window: one indirect DMA gathers the 128 source rows [128, 128];
    a selection matrix sel[e, j] = (rel_dst[e] == j) * invcnt[e] is built
    with two batched DVE ops; PE accumulates g.T @ sel into the PSUM group,
    yielding the transposed scatter-mean m^T [128 feat, 512 dst] directly.
  - Dense stage: nb^T = Wl_bb.T @ m_bb^T + Wl_sb.T @ m_sb^T + Wr.T @ x^T,
    then bias + leaky-relu fused on the scalar engine. Head (layer 2) is one
    more matmul with Wh^T producing y^T [8, dst].
"""
import os
import time
import numpy as np

import concourse.bass as bass
import concourse.bacc as bacc
import concourse.mybir as mybir
import concourse.tile as tile
from concourse.bass_utils import run_bass_kernel_spmd

P = 128
D = 128
NCORES = 8
GROUP = 512       # psum columns per accumulation group
S = 128           # max dst-column span per 128-edge window
BUCK = 25000      # src rows per int16 gather bucket
NB, NS = 100000, 50000
NLB, NLS = NB // NCORES, NS // NCORES   # 12500, 6250


# ---------------------------------------------------------------- host prep
def _shard_edges(src, dst, n_dst):
    """Split edges by dst core; per core return (src, dst_local) dst-sorted."""
    core = dst % NCORES
    loc = dst // NCORES
    out = []
    for c in range(NCORES):
        m = core == c
        s, d = src[m], loc[m]
        o = np.argsort(d, kind="stable")
        out.append((s[o].astype(np.int64), d[o].astype(np.int64)))
    return out


def _pack_type(per_core, n_loc, n_src):
    """Bucketed uniform-across-cores window packing for dma_gather.

    Edges are split by src bucket (BUCK rows each, int16-addressable); per
    (group, bucket) windows advance on a column grid uniform across cores.
    Returns (idx16 per bucket: list of [NCORES, 128, cols_b],
             rel [NCORES, P, Wtot], invc [NCORES, P, Wtot],
             groups: per group list of (bucket, k_local, col_off, span),
             gb_meta: per group dict bucket -> (idx_col_base, Nk))."""
    nbuck = (n_src + BUCK - 1) // BUCK
    ngroups = (n_loc + GROUP - 1) // GROUP
    # per (core, bucket): dst-sorted edge arrays + prefix counts
    pcb = [[None] * nbuck for _ in range(NCORES)]
    cumb = [[None] * nbuck for _ in range(NCORES)]
    counts_all = []
    for cc, (s, d) in enumerate(per_core):
        counts_all.append(np.bincount(d, minlength=n_loc))
        for b in range(nbuck):
            m = (s >= b * BUCK) & (s < (b + 1) * BUCK)
            sb_, db_ = s[m], d[m]
            pcb[cc][b] = (sb_ - b * BUCK, db_)
            cnt = np.bincount(db_, minlength=n_loc)
            cumb[cc][b] = np.concatenate([[0], np.cumsum(cnt)])
    invc_dst = [1.0 / np.maximum(c, 1) for c in counts_all]

    groups, gb_meta = [], []
    rel_cols = [[] for _ in range(NCORES)]
    invc_cols = [[] for _ in range(NCORES)]
    idx_flat = [[[] for _ in range(nbuck)] for _ in range(NCORES)]
    idx_base = [0] * nbuck
    for g in range(ngroups):
        c0, c1 = g * GROUP, min((g + 1) * GROUP, n_loc)
        wins, meta = [], {}
        for b in range(nbuck):
            k_local = 0
            c = c0
            while c < c1:
                span = min(S, c1 - c)
                while span > 1:
                    ok = all(cumb[cc][b][c + span] - cumb[cc][b][c] <= P
                             for cc in range(NCORES))
                    if ok:
                        break
                    span -= 1
                for cc in range(NCORES):
                    s_arr, d_arr = pcb[cc][b]
                    a2, b2 = cumb[cc][b][c], cumb[cc][b][c + span]
                    n = b2 - a2
                    assert n <= P
                    icol = np.zeros(P, np.int16)
                    rcol = np.full(P, -1.0, np.float32)
                    vcol = np.zeros(P, np.float32)
                    icol[:n] = s_arr[a2:b2].astype(np.int16)
                    rcol[:n] = (d_arr[a2:b2] - c).astype(np.float32)
                    vcol[:n] = invc_dst[cc][d_arr[a2:b2]].astype(np.float32)
                    idx_flat[cc][b].append(icol)
                    rel_cols[cc].append(rcol)
                    invc_cols[cc].append(vcol)
                wins.append((b, k_local, c - c0, span))
                k_local += 1
                c += span
            if k_local:
                meta[b] = (idx_base[b], k_local * P)
                idx_base[b] += k_local * P
        groups.append(wins)
        gb_meta.append(meta)

    # int16 device layout per bucket: flat i at [i%16, i//16], tiled 8x down
    idx16 = []
    for b in range(nbuck):
        per_core_arr = []
        for cc in range(NCORES):
            flat = np.concatenate(idx_flat[cc][b]) if idx_flat[cc][b] else np.zeros(0, np.int16)
            blk = flat.reshape(-1, 16).T          # [16, cols]
            per_core_arr.append(np.tile(blk, (8, 1)))
        idx16.append(np.stack(per_core_arr).astype(np.int16))
    rel = np.stack([np.stack(cols, 1) for cols in rel_cols]).astype(np.float32)
    invc = np.stack([np.stack(cols, 1) for cols in invc_cols]).astype(np.float32)
    return idx16, rel, invc, groups, gb_meta


# ------------------------------------------------------------- device build
def _build_launch(cfg):
    """Build one layer's SPMD program. cfg keys:
      tabs: {name: nrows} gather tables
      types: list of dicts(name, tab, Wtot, groups, n_loc)
      head: bool — add 8-head output (layer 2)
      out_s: bool — emit s-node output (layer 1)
    """
    nc = bacc.Bacc("TRN2", target_bir_lowering=False, debug=False,
                   num_devices=NCORES)
    f32, i32 = mybir.dt.float32, mybir.dt.int32

    i16 = mybir.dt.int16
    d_tab = {}
    for k, n in cfg["tabs"].items():
        nb_ = (n + BUCK - 1) // BUCK
        d_tab[k] = [nc.dram_tensor(f"{k}_{b}", [min(BUCK, n - b * BUCK), D], f32,
                                   kind="ExternalInput") for b in range(nb_)]
    d_xbT = nc.dram_tensor("xbT", [P, NLB], f32, kind="ExternalInput")
    d_xsT = (nc.dram_tensor("xsT", [P, NLS], f32, kind="ExternalInput")
             if cfg["out_s"] else None)
    # packed weights: Wl_bb | Wl_sb | Wr_b | [Wl_bs | Wr_s] | WhT | iota | biases
    nw = 3 * D + (2 * D if cfg["out_s"] else 0) + (8 if cfg["head"] else 0) + S + 3
    d_w = nc.dram_tensor("wts", [P, nw], f32, kind="ExternalInput")
    d_et = {}
    for t in cfg["types"]:
        W = t["Wtot"]
        d_et[t["name"]] = (
            [nc.dram_tensor(f'idx_{t["name"]}_{b}', [P, max(t["bcols"][b], 16)],
                            i16, kind="ExternalInput")
             for b in range(len(t["bcols"]))],
            nc.dram_tensor(f'rel_{t["name"]}', [P, W], f32, kind="ExternalInput"),
            nc.dram_tensor(f'ivc_{t["name"]}', [P, W], f32, kind="ExternalInput"),
        )
    d_nbT = nc.dram_tensor("nbT", [P, NLB], f32, kind="ExternalOutput")
    d_nsT = (nc.dram_tensor("nsT", [P, NLS], f32, kind="ExternalOutput")
             if cfg["out_s"] else None)
    d_yT = (nc.dram_tensor("yT", [8, NLB], f32, kind="ExternalOutput")
            if cfg["head"] else None)

    types = {t["name"]: t for t in cfg["types"]}

    from contextlib import ExitStack
    with tile.TileContext(nc) as tc, ExitStack() as ctx:
        wpool = ctx.enter_context(tc.tile_pool(name="w", bufs=1))
        gpool = ctx.enter_context(tc.tile_pool(name="g", bufs=5))
        selpool = ctx.enter_context(tc.tile_pool(name="sel", bufs=2))
        mpool = ctx.enter_context(tc.tile_pool(name="m", bufs=3))
        spool = ctx.enter_context(tc.tile_pool(name="s", bufs=3))
        appool = ctx.enter_context(tc.tile_pool(name="ap", bufs=3, space="PSUM"))
        s2pool = ctx.enter_context(tc.tile_pool(name="s2", bufs=2, space="PSUM"))
        hpool = (ctx.enter_context(tc.tile_pool(name="h", bufs=2, space="PSUM"))
                 if cfg["head"] else None)

        t_w = wpool.tile([P, nw], f32)
        nc.sync.dma_start(t_w[:], d_w[:])
        off = 0
        w_Wlbb = t_w[:, off:off + D]; off += D
        w_Wlsb = t_w[:, off:off + D]; off += D
        w_Wrb = t_w[:, off:off + D]; off += D
        if cfg["out_s"]:
            w_Wlbs = t_w[:, off:off + D]; off += D
            w_Wrs = t_w[:, off:off + D]; off += D
        if cfg["head"]:
            w_WhT = t_w[:, off:off + 8]; off += 8
        w_iota = t_w[:, off:off + S]; off += S
        w_bb = t_w[:, off:off + 1]; off += 1
        w_bs = t_w[:, off:off + 1]; off += 1
        w_bh = t_w[:, off:off + 1]; off += 1

        def aggregate(tname, g, wbase):
            """Aggregate one group of `tname` into a PSUM tile."""
            t = types[tname]
            d_idxb, d_rel, d_ivc = d_et[tname]
            wins = t["groups"][g]        # (bucket, k_local, col_off, span)
            meta = t["gb_meta"][g]       # bucket -> (slot_base, Nk)
            Wg = len(wins)
            ncols = max(c + s for (_, _, c, s) in wins)
            t_rel = mpool.tile([P, Wg], f32, tag="rel")
            nc.sync.dma_start(t_rel[:], d_rel[:, wbase:wbase + Wg])
            t_ivc = mpool.tile([P, Wg], f32, tag="ivc")
            nc.sync.dma_start(t_ivc[:], d_ivc[:, wbase:wbase + Wg])
            gtiles = {}
            for b, (sbase, Nk) in sorted(meta.items()):
                t_idx = mpool.tile([P, Nk // 16], mybir.dt.int16, tag="idx")
                nc.sync.dma_start(
                    t_idx[:], d_idxb[b][:, sbase // 16:(sbase + Nk) // 16])
                t_gb = gpool.tile([P, (Nk // P) * D], f32, tag="gb")
                nc.gpsimd.dma_gather(
                    out_ap=t_gb[:].rearrange("p (k d) -> p k d", k=Nk // P),
                    in_ap=d_tab[t["tab"]][b][:], idxs_ap=t_idx[:],
                    num_idxs=Nk, num_idxs_reg=Nk, elem_size=D,
                    single_packet=False)
                gtiles[b] = t_gb
            t_sel = selpool.tile([P, Wg * S], f32, tag="sel")
            sel3 = t_sel[:].rearrange("p (w s) -> p w s", w=Wg)
            nc.vector.tensor_tensor(
                out=sel3, in0=t_rel[:, :, None].to_broadcast([P, Wg, S]),
                in1=w_iota[:, None, :].to_broadcast([P, Wg, S]),
                op=mybir.AluOpType.is_equal)
            nc.vector.tensor_tensor(
                out=sel3, in0=sel3,
                in1=t_ivc[:, :, None].to_broadcast([P, Wg, S]),
                op=mybir.AluOpType.mult)
            t_ps = appool.tile([P, GROUP], f32, space="PSUM", tag="agg")
            for w, (b, k, coff, span) in enumerate(wins):
                nc.tensor.matmul(
                    t_ps[:, coff:coff + span],
                    lhsT=gtiles[b][:, k * D:(k + 1) * D],
                    rhs=t_sel[:, w * S:w * S + span],
                    start=(w == 0), stop=(w == Wg - 1))
            t_m = spool.tile([P, GROUP], f32, tag="mT")
            nc.vector.tensor_copy(out=t_m[:, :ncols], in_=t_ps[:, :ncols])
            return t_m, ncols

        # ---- b-node groups
        ngb = len(types["bb"]["groups"])
        ngs_on_b = len(types["sb"]["groups"])
        wb_bb = 0
        wb_sb = 0
        for g in range(ngb):
            m_bb, ncols = aggregate("bb", g, wb_bb)
            wb_bb += len(types["bb"]["groups"][g])
            has_sb = g < ngs_on_b
            if has_sb:
                m_sb, ncols_sb = aggregate("sb", g, wb_sb)
                wb_sb += len(types["sb"]["groups"][g])
            t_x = spool.tile([P, GROUP], f32, tag="xg")
            nc.sync.dma_start(t_x[:, :ncols],
                              d_xbT[:, g * GROUP:g * GROUP + ncols])
            ps2 = s2pool.tile([P, GROUP], f32, space="PSUM", tag="s2")
            nc.tensor.matmul(ps2[:, :ncols], lhsT=w_Wlbb, rhs=m_bb[:, :ncols],
                             start=True, stop=False)
            if has_sb:
                nc.tensor.matmul(ps2[:, :ncols_sb], lhsT=w_Wlsb,
                                 rhs=m_sb[:, :ncols_sb],
                                 start=False, stop=False)
            nc.tensor.matmul(ps2[:, :ncols], lhsT=w_Wrb, rhs=t_x[:, :ncols],
                             start=False, stop=True)
            t_o = spool.tile([P, GROUP], f32, tag="ob")
            nc.scalar.activation(out=t_o[:, :ncols], in_=ps2[:, :ncols],
                                 func=mybir.ActivationFunctionType.Lrelu,
                                 bias=w_bb, alpha=0.01)
            nc.sync.dma_start(d_nbT[:, g * GROUP:g * GROUP + ncols],
                              t_o[:, :ncols])
            if cfg["head"]:
                ps3 = hpool.tile([8, GROUP], f32, space="PSUM", tag="hd")
                nc.tensor.matmul(ps3[:, :ncols], lhsT=w_WhT,
                                 rhs=t_o[:, :ncols], start=True, stop=True)
                t_y = spool.tile([8, GROUP], f32, tag="yt")
                nc.vector.tensor_scalar_add(t_y[:, :ncols], ps3[:, :ncols],
                                            w_bh[:8])
                nc.sync.dma_start(d_yT[:, g * GROUP:g * GROUP + ncols],
                                  t_y[:, :ncols])

        # ---- s-node groups (layer 1 only)
        if cfg["out_s"]:
            wb_bs = 0
            for g in range(len(types["bs"]["groups"])):
                m_bs, ncols = aggregate("bs", g, wb_bs)
                wb_bs += len(types["bs"]["groups"][g])
                t_x = spool.tile([P, GROUP], f32, tag="xg")
                nc.sync.dma_start(t_x[:, :ncols],
                                  d_xsT[:, g * GROUP:g * GROUP + ncols])
                ps2 = s2pool.tile([P, GROUP], f32, space="PSUM", tag="s2")
                nc.tensor.matmul(ps2[:, :ncols], lhsT=w_Wlbs,
                                 rhs=m_bs[:, :ncols], start=True, stop=False)
                nc.tensor.matmul(ps2[:, :ncols], lhsT=w_Wrs,
                                 rhs=t_x[:, :ncols], start=False, stop=True)
                t_o = spool.tile([P, GROUP], f32, tag="ob")
                nc.scalar.activation(out=t_o[:, :ncols], in_=ps2[:, :ncols],
                                     func=mybir.ActivationFunctionType.Lrelu,
                                     bias=w_bs, alpha=0.01)
                nc.sync.dma_start(d_nsT[:, g * GROUP:g * GROUP + ncols],
                                  t_o[:, :ncols])

    nc.compile()
    return nc


def _pack_weights(cfg, Wlbb, Wlsb, Wrb, bb, bs_bias=None, Wlbs=None, Wrs=None,
                  WhT=None, bh0=None):
    nw = 3 * D + (2 * D if cfg["out_s"] else 0) + (8 if cfg["head"] else 0) + S + 3
    w = np.zeros((P, nw), np.float32)
    off = 0
    for M in [Wlbb, Wlsb, Wrb]:
        w[:, off:off + D] = M; off += D
    if cfg["out_s"]:
        w[:, off:off + D] = Wlbs; off += D
        w[:, off:off + D] = Wrs; off += D
    if cfg["head"]:
        w[:, off:off + 8] = WhT; off += 8
    w[:, off:off + S] = np.arange(S, dtype=np.float32)[None, :]; off += S
    w[:, off] = bb; off += 1
    if bs_bias is not None:
        w[:, off] = bs_bias
    off += 1
    if bh0 is not None:
        w[:8, off] = bh0
    return w


LAST_HW_NS = None
LAST_EXEC_S = None


def kernel(x_b, x_s, Wl, bl, Wr, Wh, bh, ei_bb, ei_sb, ei_bs):
    x_b = np.asarray(x_b, np.float32); x_s = np.asarray(x_s, np.float32)
    Wl = np.asarray(Wl, np.float32); bl = np.asarray(bl, np.float32)
    Wr = np.asarray(Wr, np.float32); Wh = np.asarray(Wh, np.float32)
    bh = np.asarray(bh, np.float32)
    ei_bb = np.asarray(ei_bb); ei_sb = np.asarray(ei_sb); ei_bs = np.asarray(ei_bs)

    # ---------------- layer 1 prep (original node ids as gather indices)
    pc_bb = _shard_edges(ei_bb[0], ei_bb[1], NB)
    pc_sb = _shard_edges(ei_sb[0], ei_sb[1], NB)   # dst are b-nodes < NS
    pc_bs = _shard_edges(ei_bs[0], ei_bs[1], NS)
    i_bb, r_bb, v_bb, g_bb, m_bb = _pack_type(pc_bb, NLB, NB)
    i_sb, r_sb, v_sb, g_sb, m_sb = _pack_type(pc_sb, NS // NCORES, NS)
    i_bs, r_bs, v_bs, g_bs, m_bs = _pack_type(pc_bs, NLS, NS)

    cfgA = {
        "tabs": {"tab_b": NB, "tab_s": NS},
        "types": [
            {"name": "bb", "tab": "tab_b", "Wtot": r_bb.shape[2], "groups": g_bb,
             "gb_meta": m_bb, "bcols": [a.shape[2] for a in i_bb]},
            {"name": "sb", "tab": "tab_s", "Wtot": r_sb.shape[2], "groups": g_sb,
             "gb_meta": m_sb, "bcols": [a.shape[2] for a in i_sb]},
            {"name": "bs", "tab": "tab_b", "Wtot": r_bs.shape[2], "groups": g_bs,
             "gb_meta": m_bs, "bcols": [a.shape[2] for a in i_bs]},
        ],
        "head": False, "out_s": True,
    }
    ncA = _build_launch(cfgA)
    wA = _pack_weights(cfgA, Wl[0, 0], Wl[0, 1], Wr[0, 0] + Wr[0, 1],
                       bl[0, 0] + bl[0, 1], bs_bias=bl[0, 2],
                       Wlbs=Wl[0, 2], Wrs=Wr[0, 2])
    def tab_splits(tab):
        return {f"{n}_{b}": np.ascontiguousarray(tab[b * BUCK:(b + 1) * BUCK])
                for b in range((tab.shape[0] + BUCK - 1) // BUCK)
                for n in [None]}

    def bucket_ins(name, arrs, c):
        return {f"{name}_{b}": (a[c] if a.shape[2] >= 16 else
                                np.zeros((P, 16), np.int16))
                for b, a in enumerate(arrs)}

    tb = {f"tab_b_{b}": np.ascontiguousarray(x_b[b * BUCK:(b + 1) * BUCK])
          for b in range(4)}
    tsp = {f"tab_s_{b}": np.ascontiguousarray(x_s[b * BUCK:(b + 1) * BUCK])
           for b in range(2)}
    in_maps = []
    for c in range(NCORES):
        in_maps.append({
            **tb, **tsp,
            "xbT": np.ascontiguousarray(x_b[c::NCORES].T),
            "xsT": np.ascontiguousarray(x_s[c::NCORES].T),
            "wts": wA,
            **bucket_ins("idx_bb", i_bb, c), "rel_bb": r_bb[c], "ivc_bb": v_bb[c],
            **bucket_ins("idx_sb", i_sb, c), "rel_sb": r_sb[c], "ivc_sb": v_sb[c],
            **bucket_ins("idx_bs", i_bs, c), "rel_bs": r_bs[c], "ivc_bs": v_bs[c],
        })
    _tr = False
    _t0 = time.time()
    resA = run_bass_kernel_spmd(ncA, in_maps, core_ids=list(range(NCORES)),
                                trace=_tr, trace_cores=[0] if _tr else None)
    _execA = time.time() - _t0
    if _tr:
        print("launchA exec_ns:", resA.exec_time_ns,
              "trace:", (resA.instructions_and_trace or (None, None))[1], flush=True)
    nbT = [resA.results[c]["nbT"] for c in range(NCORES)]
    nsT = [resA.results[c]["nsT"] for c in range(NCORES)]

    # ---------------- layer 2: host halo exchange + index translation
    xb1 = np.concatenate([t.T for t in nbT], 0)   # [NB, D] core-block order
    xs1 = np.concatenate([t.T for t in nsT], 0)   # [NS, D]

    def tr_b(v):
        return (v % NCORES) * NLB + v // NCORES

    def tr_s(v):
        return (v % NCORES) * NLS + v // NCORES

    pc_bb2 = _shard_edges(tr_b(ei_bb[0]), ei_bb[1], NB)
    pc_sb2 = _shard_edges(tr_s(ei_sb[0]), ei_sb[1], NB)
    i_bb2, r_bb2, v_bb2, g_bb2, m_bb2 = _pack_type(pc_bb2, NLB, NB)
    i_sb2, r_sb2, v_sb2, g_sb2, m_sb2 = _pack_type(pc_sb2, NS // NCORES, NS)

    cfgB = {
        "tabs": {"tab_b": NB, "tab_s": NS},
        "types": [
            {"name": "bb", "tab": "tab_b", "Wtot": r_bb2.shape[2], "groups": g_bb2,
             "gb_meta": m_bb2, "bcols": [a.shape[2] for a in i_bb2]},
            {"name": "sb", "tab": "tab_s", "Wtot": r_sb2.shape[2], "groups": g_sb2,
             "gb_meta": m_sb2, "bcols": [a.shape[2] for a in i_sb2]},
        ],
        "head": True, "out_s": False,
    }
    ncB = _build_launch(cfgB)
    wB = _pack_weights(cfgB, Wl[1, 0], Wl[1, 1], Wr[1, 0] + Wr[1, 1],
                       bl[1, 0] + bl[1, 1], WhT=Wh.T, bh0=bh)
    tb1 = {f"tab_b_{b}": np.ascontiguousarray(xb1[b * BUCK:(b + 1) * BUCK])
           for b in range(4)}
    ts1 = {f"tab_s_{b}": np.ascontiguousarray(xs1[b * BUCK:(b + 1) * BUCK])
           for b in range(2)}
    in_mapsB = []
    for c in range(NCORES):
        in_mapsB.append({
            **tb1, **ts1,
            "xbT": nbT[c], "wts": wB,
            **bucket_ins("idx_bb", i_bb2, c), "rel_bb": r_bb2[c], "ivc_bb": v_bb2[c],
            **bucket_ins("idx_sb", i_sb2, c), "rel_sb": r_sb2[c], "ivc_sb": v_sb2[c],
        })
    _t0 = time.time()
    resB = run_bass_kernel_spmd(ncB, in_mapsB, core_ids=list(range(NCORES)),
                                trace=_tr, trace_cores=[0] if _tr else None)
    _execB = time.time() - _t0
    if _tr:
        print("launchB exec_ns:", resB.exec_time_ns,
              "trace:", (resB.instructions_and_trace or (None, None))[1], flush=True)
    global LAST_HW_NS, LAST_EXEC_S
    if resA.exec_time_ns and resB.exec_time_ns:
        LAST_HW_NS = int(resA.exec_time_ns) + int(resB.exec_time_ns)
    LAST_EXEC_S = (_execA, _execB)

    y = np.empty((NB, 8), np.float32)
    for c in range(NCORES):
        y[np.arange(NLB) * NCORES + c] = resB.results[c]["yT"].T
    return y



# revision 4
# speedup vs baseline: 89.9297x; 89.9297x over previous
"""HGNN (2-layer hetero GraphSAGE + 8 heads) on 8 trn2 NeuronCores.

Single fused SPMD launch. Nodes are dst-interleaved (core = v % 8,
local = v // 8); each core receives only its node shard (bf16) plus edge
window metadata. On device:

  1. AllGather the x_b / x_s shards into full gather tables (core-block
     row order; gather indices are pre-translated on host).
  2. Layer 1: per 512-dst-column PSUM group, 128-edge windows (sorted by
     dst, cut on a column grid uniform across cores) are gathered with
     indirect DMA; a 0/1 selection matrix sel[e, j] = (rel[e] == j) built
     by one DVE is_equal feeds PE accumulation g.T @ sel -> raw sums s^T;
     the scatter-mean divides by per-dst counts via a DMA-broadcast
     1/cnt row. Dense stage + leaky-relu as usual; outputs are PE-
     transposed back to node-major and AllGathered into layer-2 tables.
  3. Layer 2 reuses the *same* window metadata (same edges, same table
     layout) against the layer-1 tables, then applies the 8-head
     classifier -> yT [8, NLB] per core (the only launch output).

kernel() runs one warm-up launch (hits the persistent jax compilation
cache) and then one timed launch; LAST_EXEC_S reports the timed
device-launch wall.
"""
import os
import time
import numpy as np

import jax
jax.config.update("jax_compilation_cache_dir",
                  os.path.expanduser("~/.cache/hgnn_jaxcache"))
jax.config.update("jax_persistent_cache_min_entry_size_bytes", -1)
jax.config.update("jax_persistent_cache_min_compile_time_secs", 0.0)

import ml_dtypes
import concourse.bass as bass
import concourse.bacc as bacc
import concourse.mybir as mybir
import concourse.tile as tile
from concourse.bass_utils import run_bass_kernel_spmd

P = 128
D = 128
NCORES = 8
GROUP = 512       # psum columns per accumulation group
S = 128           # max dst-column span per 128-edge window
BUCK = 25000      # src table rows per int16 gather bucket
NB, NS = 100000, 50000
NLB, NLS = NB // NCORES, NS // NCORES   # 12500, 6250
BF16 = ml_dtypes.bfloat16


# ---------------------------------------------------------------- host prep
def _tr(v, nl):
    """Global node id -> row in the core-block AllGather table."""
    return (v % NCORES) * nl + v // NCORES


def _prep_type(src_t, dst, n_tab, n_loc):
    """Shard edges by dst core and pack 128-edge windows on a column grid
    uniform across cores (min-over-cores advance), bucketed by src table
    row so gather indices fit int16.

    src_t: edge source *table rows* (already translated), dst: global dst.
    Returns (idx16: per bucket [NCORES, 16, cols] int16,
             rel   [NCORES, P, Wtot] int8 (-1 pad),
             invc  [NCORES, n_loc] f32,
             groups: per group list of (bucket, k_local, col_off, span),
             gb_meta: per group dict bucket -> (idx slot base, Nk))."""
    nbuck = n_tab // BUCK
    ngroups = -(-n_loc // GROUP)
    core = dst % NCORES
    loc = dst // NCORES
    pcb = [[None] * nbuck for _ in range(NCORES)]
    cumb = [[None] * nbuck for _ in range(NCORES)]
    invc = np.empty((NCORES, n_loc), np.float32)
    for cc in range(NCORES):
        m = core == cc
        s, d = src_t[m], loc[m]
        o = np.argsort(d, kind="stable")
        s, d = s[o], d[o]
        invc[cc] = 1.0 / np.maximum(np.bincount(d, minlength=n_loc), 1)
        for b in range(nbuck):
            mb = (s >= b * BUCK) & (s < (b + 1) * BUCK)
            pcb[cc][b] = (s[mb] - b * BUCK, d[mb])
            cntb = np.bincount(d[mb], minlength=n_loc)
            cumb[cc][b] = np.concatenate([[0], np.cumsum(cntb)])

    groups, gb_meta = [], []
    rel_cols = [[] for _ in range(NCORES)]
    idx_flat = [[[] for _ in range(nbuck)] for _ in range(NCORES)]
    idx_base = [0] * nbuck
    for g in range(ngroups):
        c0, c1 = g * GROUP, min((g + 1) * GROUP, n_loc)
        wins, meta = [], {}
        for b in range(nbuck):
            k_local = 0
            c = c0
            while c < c1:
                span = min(S, c1 - c)
                while span > 1:
                    ok = all(cumb[cc][b][c + span] - cumb[cc][b][c] <= P
                             for cc in range(NCORES))
                    if ok:
                        break
                    span -= 1
                for cc in range(NCORES):
                    s_arr, d_arr = pcb[cc][b]
                    a2, b2 = cumb[cc][b][c], cumb[cc][b][c + span]
                    n = b2 - a2
                    assert n <= P
                    icol = np.zeros(P, np.int16)
                    rcol = np.full(P, -1, np.int8)
                    icol[:n] = s_arr[a2:b2].astype(np.int16)
                    rcol[:n] = (d_arr[a2:b2] - c).astype(np.int8)
                    idx_flat[cc][b].append(icol)
                    rel_cols[cc].append(rcol)
                wins.append((b, k_local, c - c0, span))
                k_local += 1
                c += span
            if k_local:
                meta[b] = (idx_base[b], k_local * P)
                idx_base[b] += k_local * P
        groups.append(wins)
        gb_meta.append(meta)

    idx16 = []
    for b in range(nbuck):
        per_core = []
        for cc in range(NCORES):
            flat = (np.concatenate(idx_flat[cc][b]) if idx_flat[cc][b]
                    else np.zeros(256, np.int16))
            per_core.append(np.ascontiguousarray(flat.reshape(-1, 16).T))
        idx16.append(np.stack(per_core))                 # [NCORES, 16, cols]
    rel = np.stack([np.stack(cs, 1) for cs in rel_cols]).astype(np.int8)
    return idx16, rel, invc, groups, gb_meta


# ------------------------------------------------------------- device build
def _build(types):
    """types: dict name -> dict(bcols, Wtot, groups, gb_meta, tab ('b'|'s'))."""
    nc = bacc.Bacc("TRN2", target_bir_lowering=False, debug=False,
                   num_devices=NCORES)
    f32, bf16 = mybir.dt.float32, mybir.dt.bfloat16
    i16, i8, i32 = mybir.dt.int16, mybir.dt.int8, mybir.dt.int32

    d_xb = nc.dram_tensor("xb", [NLB, P], bf16, kind="ExternalInput")
    d_xs = nc.dram_tensor("xs", [NLS, P], bf16, kind="ExternalInput")
    # 8 stacked [D, D] bf16 mats + WhT [D, 8]:
    # Wlbb0 Wlsb0 Wrb0 Wlbs0 Wrs0 Wlbb1 Wlsb1 Wrb1 WhT
    d_wb = nc.dram_tensor("wb", [P, 8 * D + 8], bf16, kind="ExternalInput")
    # bias columns f32: bb0 bs0 bb1 bh(first 8 rows)
    d_bias = nc.dram_tensor("bias", [P, 4], f32, kind="ExternalInput")
    d_iv = {"bb": nc.dram_tensor("ivb", [1, NLB], f32, kind="ExternalInput"),
            "sb": nc.dram_tensor("ivs", [1, NLB], f32, kind="ExternalInput"),
            "bs": nc.dram_tensor("ivq", [1, NLS], f32, kind="ExternalInput")}
    d_idx = {t: [nc.dram_tensor(f"idx_{t}_{b}", [16, c], i16,
                                kind="ExternalInput")
                 for b, c in enumerate(types[t]["bcols"])] for t in types}
    d_rel = {t: nc.dram_tensor(f"rel_{t}", [P, types[t]["Wtot"]], i8,
                               kind="ExternalInput") for t in types}
    d_yT = nc.dram_tensor("yT", [8, NLB], f32, kind="ExternalOutput")

    from contextlib import ExitStack
    with tile.TileContext(nc) as tc, ExitStack() as ctx:
        wpool = ctx.enter_context(tc.tile_pool(name="w", bufs=1))
        dpool = ctx.enter_context(tc.tile_pool(name="dr", bufs=1, space="DRAM"))
        gpool = ctx.enter_context(tc.tile_pool(name="g", bufs=6))
        selpool = ctx.enter_context(tc.tile_pool(name="sel", bufs=2))
        mpool = ctx.enter_context(tc.tile_pool(name="m", bufs=2))
        spool = ctx.enter_context(tc.tile_pool(name="s", bufs=3))
        appool = ctx.enter_context(tc.tile_pool(name="ap", bufs=3, space="PSUM"))
        s2pool = ctx.enter_context(tc.tile_pool(name="s2", bufs=2, space="PSUM"))
        trpool = ctx.enter_context(tc.tile_pool(name="tr", bufs=2, space="PSUM"))
        hpool = ctx.enter_context(tc.tile_pool(name="h", bufs=1, space="PSUM"))

        # ---- DRAM scratch: bounce shards, gather tables, layer-1 staging
        bounce_b = dpool.tile([NLB, P], bf16, tag="bnb")
        bounce_s = dpool.tile([NLS, P], bf16, tag="bns")
        tab_b0 = dpool.tile([NB, P], bf16, tag="tb0")
        tab_s0 = dpool.tile([NS, P], bf16, tag="ts0")
        tab_b1 = dpool.tile([NB, P], bf16, tag="tb1")
        tab_s1 = dpool.tile([NS, P], bf16, tag="ts1")
        nb_nm = dpool.tile([NLB, P], bf16, tag="nbm")   # L1 b out, node-major
        ns_nm = dpool.tile([NLS, P], bf16, tag="nsm")
        d_nbT = dpool.tile([P, NLB], bf16, tag="nbt")   # L1 b out, feat-major

        grp = [list(range(NCORES))]
        nc.sync.dma_start(bounce_b[:], d_xb[:])
        nc.gpsimd.collective_compute(
            "AllGather", mybir.AluOpType.bypass, replica_groups=grp,
            ins=[bounce_b[:].opt()], outs=[tab_b0[:].opt()])
        nc.sync.dma_start(bounce_s[:], d_xs[:])
        nc.gpsimd.collective_compute(
            "AllGather", mybir.AluOpType.bypass, replica_groups=grp,
            ins=[bounce_s[:].opt()], outs=[tab_s0[:].opt()])

        # ---- constants: weights, iota row, identity
        t_w = wpool.tile([P, 8 * D + 8], bf16, tag="wb")
        nc.sync.dma_start(t_w[:], d_wb[:])
        wm = {n: t_w[:, i * D:(i + 1) * D] for i, n in enumerate(
            ["Wlbb0", "Wlsb0", "Wrb0", "Wlbs0", "Wrs0",
             "Wlbb1", "Wlsb1", "Wrb1"])}
        w_WhT = t_w[:, 8 * D:8 * D + 8]
        t_bias = wpool.tile([P, 4], f32, tag="bias")
        nc.sync.dma_start(t_bias[:], d_bias[:])
        b_bb0, b_bs0, b_bb1 = (t_bias[:, i:i + 1] for i in range(3))
        b_h = t_bias[:8, 3:4]

        t_ii = wpool.tile([P, S], i32, tag="ii")
        nc.gpsimd.iota(t_ii[:], pattern=[[1, S]], base=0, channel_multiplier=0)
        t_iota = wpool.tile([P, S], f32, tag="iota")
        nc.vector.tensor_copy(out=t_iota[:], in_=t_ii[:])
        t_ip = wpool.tile([P, 1], i32, tag="ip")
        nc.gpsimd.iota(t_ip[:], pattern=[[0, 1]], base=0, channel_multiplier=1)
        t_ipf = wpool.tile([P, 1], f32, tag="ipf")
        nc.vector.tensor_copy(out=t_ipf[:], in_=t_ip[:])
        t_id = wpool.tile([P, P], bf16, tag="ident")
        nc.vector.tensor_tensor(out=t_id[:], in0=t_iota[:],
                                in1=t_ipf[:].to_broadcast([P, P]),
                                op=mybir.AluOpType.is_equal)

        # ---- resident per-type idx (replicated 16->128 on device) and rel f32
        t_idx, t_rel = {}, {}
        for t in types:
            t_idx[t] = []
            for b, cols in enumerate(types[t]["bcols"]):
                ti = wpool.tile([P, cols], i16, tag=f"ix_{t}_{b}")
                for k in range(8):
                    nc.sync.dma_start(ti[16 * k:16 * (k + 1), :], d_idx[t][b][:])
                t_idx[t].append(ti)
            tr8 = wpool.tile([P, types[t]["Wtot"]], i8, tag=f"r8_{t}")
            nc.sync.dma_start(tr8[:], d_rel[t][:])
            trf = wpool.tile([P, types[t]["Wtot"]], f32, tag=f"rf_{t}")
            nc.vector.tensor_copy(out=trf[:], in_=tr8[:])
            t_rel[t] = trf

        def aggregate(tname, g, wbase, tab):
            """Accumulate one group's scatter-sum into PSUM: returns
            (psum tile [P, GROUP] f32, ncols)."""
            ty = types[tname]
            wins = ty["groups"][g]
            meta = ty["gb_meta"][g]
            Wg = len(wins)
            ncols = max(c + sp for (_, _, c, sp) in wins)
            t_sel = selpool.tile([P, Wg * S], bf16, tag="sel")
            sel3 = t_sel[:].rearrange("p (w s) -> p w s", w=Wg)
            rel = t_rel[tname]
            nc.vector.tensor_tensor(
                out=sel3,
                in0=rel[:, wbase:wbase + Wg, None].to_broadcast([P, Wg, S]),
                in1=t_iota[:, None, :].to_broadcast([P, Wg, S]),
                op=mybir.AluOpType.is_equal)
            gtiles = {}
            for b, (sbase, Nk) in sorted(meta.items()):
                t_gb = gpool.tile([P, (Nk // P) * D], mybir.dt.bfloat16,
                                  tag="gb")
                nc.gpsimd.dma_gather(
                    out_ap=t_gb[:].rearrange("p (k d) -> p k d", k=Nk // P),
                    in_ap=tab[b * BUCK:(b + 1) * BUCK, :],
                    idxs_ap=t_idx[tname][b][:, sbase // 16:(sbase + Nk) // 16],
                    num_idxs=Nk, num_idxs_reg=Nk, elem_size=D,
                    single_packet=False)
                gtiles[b] = t_gb
            t_ps = appool.tile([P, GROUP], mybir.dt.float32, space="PSUM",
                               tag="agg")
            for w, (b, k, coff, span) in enumerate(wins):
                nc.tensor.matmul(
                    t_ps[:, coff:coff + span],
                    lhsT=gtiles[b][:, k * D:(k + 1) * D],
                    rhs=t_sel[:, w * S:w * S + span],
                    start=(w == 0), stop=(w == Wg - 1))
            return t_ps, ncols

        def scale_mean(tname, g, t_ps, ncols):
            """m^T = s^T * (1/cnt) broadcast across partitions -> bf16."""
            t_iv = spool.tile([P, GROUP], mybir.dt.float32, tag="iv")
            nc.sync.dma_start(
                t_iv[:, :ncols],
                d_iv[tname][0:1, g * GROUP:g * GROUP + ncols]
                .to_broadcast([P, ncols]))
            t_m = mpool.tile([P, GROUP], mybir.dt.bfloat16, tag=f"m_{tname}")
            nc.vector.tensor_tensor(out=t_m[:, :ncols], in0=t_ps[:, :ncols],
                                    in1=t_iv[:, :ncols],
                                    op=mybir.AluOpType.mult)
            return t_m

        def xT_blocks(src_dram, g, ncols, n_loc):
            """Load node-major rows for this group and PE-transpose into a
            feature-major [P, ncols] bf16 tile."""
            t_x = spool.tile([P, GROUP], mybir.dt.bfloat16, tag="xg")
            j0 = 0
            while j0 < ncols:
                w = min(P, ncols - j0)
                t_blk = gpool.tile([P, P], mybir.dt.bfloat16, tag="xblk")
                nc.sync.dma_start(
                    t_blk[:w, :],
                    src_dram[g * GROUP + j0:g * GROUP + j0 + w, :])
                ps_t = trpool.tile([P, P], mybir.dt.bfloat16, space="PSUM",
                                   tag="tr")
                nc.tensor.transpose(ps_t[:, :w], t_blk[:w, :], t_id[:w, :w])
                nc.vector.tensor_copy(out=t_x[:, j0:j0 + w], in_=ps_t[:, :w])
                j0 += w
            return t_x

        def emit_node_major(t_o, dst_dram, g, ncols):
            """PE-transpose feature-major output back to node-major rows."""
            j0 = 0
            while j0 < ncols:
                w = min(P, ncols - j0)
                ps_t = trpool.tile([P, P], mybir.dt.bfloat16, space="PSUM",
                                   tag="tr")
                nc.tensor.transpose(ps_t[:w, :], t_o[:, j0:j0 + w], t_id[:])
                t_nm = gpool.tile([P, P], mybir.dt.bfloat16, tag="nm")
                nc.vector.tensor_copy(out=t_nm[:w, :], in_=ps_t[:w, :])
                nc.sync.dma_start(
                    dst_dram[g * GROUP + j0:g * GROUP + j0 + w, :],
                    t_nm[:w, :])
                j0 += w

        # ---------------- layer 1, s-dst groups (first: frees tab_s1 early)
        wb_bs = 0
        for g in range(len(types["bs"]["groups"])):
            ps_agg, ncols = aggregate("bs", g, wb_bs, tab_b0)
            wb_bs += len(types["bs"]["groups"][g])
            t_m = scale_mean("bs", g, ps_agg, ncols)
            t_x = xT_blocks(d_xs, g, ncols, NLS)
            ps2 = s2pool.tile([P, GROUP], mybir.dt.float32, space="PSUM",
                              tag="s2")
            nc.tensor.matmul(ps2[:, :ncols], lhsT=wm["Wlbs0"],
                             rhs=t_m[:, :ncols], start=True, stop=False)
            nc.tensor.matmul(ps2[:, :ncols], lhsT=wm["Wrs0"],
                             rhs=t_x[:, :ncols], start=False, stop=True)
            t_o = spool.tile([P, GROUP], mybir.dt.bfloat16, tag="ob")
            nc.scalar.activation(out=t_o[:, :ncols], in_=ps2[:, :ncols],
                                 func=mybir.ActivationFunctionType.Lrelu,
                                 bias=b_bs0, alpha=0.01)
            emit_node_major(t_o, ns_nm, g, ncols)
        nc.gpsimd.collective_compute(
            "AllGather", mybir.AluOpType.bypass, replica_groups=grp,
            ins=[ns_nm[:].opt()], outs=[tab_s1[:].opt()])

        # ---------------- layer 1, b-dst groups
        wb_bb = 0
        wb_sb = 0
        for g in range(len(types["bb"]["groups"])):
            ps_bb, ncols = aggregate("bb", g, wb_bb, tab_b0)
            wb_bb += len(types["bb"]["groups"][g])
            m_bb = scale_mean("bb", g, ps_bb, ncols)
            has_sb = bool(types["sb"]["groups"][g])
            if has_sb:
                ps_sb, ncols_sb = aggregate("sb", g, wb_sb, tab_s0)
                wb_sb += len(types["sb"]["groups"][g])
                m_sb = scale_mean("sb", g, ps_sb, ncols_sb)
            t_x = xT_blocks(d_xb, g, ncols, NLB)
            ps2 = s2pool.tile([P, GROUP], mybir.dt.float32, space="PSUM",
                              tag="s2")
            nc.tensor.matmul(ps2[:, :ncols], lhsT=wm["Wlbb0"],
                             rhs=m_bb[:, :ncols], start=True, stop=False)
            if has_sb:
                nc.tensor.matmul(ps2[:, :ncols_sb], lhsT=wm["Wlsb0"],
                                 rhs=m_sb[:, :ncols_sb], start=False,
                                 stop=False)
            nc.tensor.matmul(ps2[:, :ncols], lhsT=wm["Wrb0"],
                             rhs=t_x[:, :ncols], start=False, stop=True)
            t_o = spool.tile([P, GROUP], mybir.dt.bfloat16, tag="ob")
            nc.scalar.activation(out=t_o[:, :ncols], in_=ps2[:, :ncols],
                                 func=mybir.ActivationFunctionType.Lrelu,
                                 bias=b_bb0, alpha=0.01)
            nc.sync.dma_start(d_nbT[:, g * GROUP:g * GROUP + ncols],
                              t_o[:, :ncols])
            emit_node_major(t_o, nb_nm, g, ncols)
        nc.gpsimd.collective_compute(
            "AllGather", mybir.AluOpType.bypass, replica_groups=grp,
            ins=[nb_nm[:].opt()], outs=[tab_b1[:].opt()])

        # ---------------- layer 2, b-dst groups (+ heads)
        wb_bb = 0
        wb_sb = 0
        for g in range(len(types["bb"]["groups"])):
            ps_bb, ncols = aggregate("bb", g, wb_bb, tab_b1)
            wb_bb += len(types["bb"]["groups"][g])
            m_bb = scale_mean("bb", g, ps_bb, ncols)
            has_sb = bool(types["sb"]["groups"][g])
            if has_sb:
                ps_sb, ncols_sb = aggregate("sb", g, wb_sb, tab_s1)
                wb_sb += len(types["sb"]["groups"][g])
                m_sb = scale_mean("sb", g, ps_sb, ncols_sb)
            t_x = spool.tile([P, GROUP], mybir.dt.bfloat16, tag="xg")
            nc.sync.dma_start(t_x[:, :ncols],
                              d_nbT[:, g * GROUP:g * GROUP + ncols])
            ps2 = s2pool.tile([P, GROUP], mybir.dt.float32, space="PSUM",
                              tag="s2")
            nc.tensor.matmul(ps2[:, :ncols], lhsT=wm["Wlbb1"],
                             rhs=m_bb[:, :ncols], start=True, stop=False)
            if has_sb:
                nc.tensor.matmul(ps2[:, :ncols_sb], lhsT=wm["Wlsb1"],
                                 rhs=m_sb[:, :ncols_sb], start=False,
                                 stop=False)
            nc.tensor.matmul(ps2[:, :ncols], lhsT=wm["Wrb1"],
                             rhs=t_x[:, :ncols], start=False, stop=True)
            t_o = spool.tile([P, GROUP], mybir.dt.bfloat16, tag="ob")
            nc.scalar.activation(out=t_o[:, :ncols], in_=ps2[:, :ncols],
                                 func=mybir.ActivationFunctionType.Lrelu,
                                 bias=b_bb1, alpha=0.01)
            ps3 = hpool.tile([8, GROUP], mybir.dt.float32, space="PSUM",
                             tag="hd")
            nc.tensor.matmul(ps3[:, :ncols], lhsT=w_WhT, rhs=t_o[:, :ncols],
                             start=True, stop=True)
            t_y = spool.tile([8, GROUP], mybir.dt.float32, tag="yt")
            nc.vector.tensor_scalar_add(t_y[:, :ncols], ps3[:, :ncols], b_h)
            nc.sync.dma_start(d_yT[:, g * GROUP:g * GROUP + ncols],
                              t_y[:, :ncols])

    nc.compile()
    return nc


LAST_HW_NS = None
LAST_EXEC_S = None
LAST_WARM_S = None


def kernel(x_b, x_s, Wl, bl, Wr, Wh, bh, ei_bb, ei_sb, ei_bs):
    x_b = np.asarray(x_b, np.float32)
    x_s = np.asarray(x_s, np.float32)
    Wl = np.asarray(Wl, np.float32)
    bl = np.asarray(bl, np.float32)
    Wr = np.asarray(Wr, np.float32)
    Wh = np.asarray(Wh, np.float32)
    bh = np.asarray(bh, np.float32)
    ei_bb = np.asarray(ei_bb).astype(np.int64)
    ei_sb = np.asarray(ei_sb).astype(np.int64)
    ei_bs = np.asarray(ei_bs).astype(np.int64)

    # window packing (indices pre-translated into AllGather table rows;
    # identical metadata serves both layers)
    i_bb, r_bb, v_bb, g_bb, m_bb = _prep_type(
        _tr(ei_bb[0], NLB), ei_bb[1], NB, NLB)
    i_sb, r_sb, v_sb, g_sb, m_sb = _prep_type(
        _tr(ei_sb[0], NLS), ei_sb[1], NS, NLB)
    i_bs, r_bs, v_bs, g_bs, m_bs = _prep_type(
        _tr(ei_bs[0], NLB), ei_bs[1], NB, NLS)

    types = {
        "bb": {"bcols": [a.shape[2] for a in i_bb], "Wtot": r_bb.shape[2],
               "groups": g_bb, "gb_meta": m_bb},
        "sb": {"bcols": [a.shape[2] for a in i_sb], "Wtot": r_sb.shape[2],
               "groups": g_sb, "gb_meta": m_sb},
        "bs": {"bcols": [a.shape[2] for a in i_bs], "Wtot": r_bs.shape[2],
               "groups": g_bs, "gb_meta": m_bs},
    }
    nc = _build(types)

    # weight payload (bf16) + bias columns (f32)
    wmats = [Wl[0, 0], Wl[0, 1], Wr[0, 0] + Wr[0, 1], Wl[0, 2], Wr[0, 2],
             Wl[1, 0], Wl[1, 1], Wr[1, 0] + Wr[1, 1]]
    wb_np = np.zeros((P, 8 * D + 8), BF16)
    for i, M in enumerate(wmats):
        wb_np[:, i * D:(i + 1) * D] = M.astype(BF16)
    wb_np[:, 8 * D:8 * D + 8] = Wh.T.astype(BF16)
    bias_np = np.zeros((P, 4), np.float32)
    bias_np[:, 0] = bl[0, 0] + bl[0, 1]
    bias_np[:, 1] = bl[0, 2]
    bias_np[:, 2] = bl[1, 0] + bl[1, 1]
    bias_np[:8, 3] = bh

    in_maps = []
    for c in range(NCORES):
        im = {
            "xb": np.ascontiguousarray(x_b[c::NCORES]).astype(BF16),
            "xs": np.ascontiguousarray(x_s[c::NCORES]).astype(BF16),
            "wb": wb_np, "bias": bias_np,
            "ivb": v_bb[c][None, :], "ivs": v_sb[c][None, :],
            "ivq": v_bs[c][None, :],
            "rel_bb": r_bb[c], "rel_sb": r_sb[c], "rel_bs": r_bs[c],
        }
        for t, arrs in (("bb", i_bb), ("sb", i_sb), ("bs", i_bs)):
            for b, a in enumerate(arrs):
                im[f"idx_{t}_{b}"] = a[c]
        in_maps.append(im)

    global LAST_HW_NS, LAST_EXEC_S, LAST_WARM_S
    t0 = time.time()
    run_bass_kernel_spmd(nc, in_maps, core_ids=list(range(NCORES)))
    LAST_WARM_S = time.time() - t0

    t0 = time.time()
    res = run_bass_kernel_spmd(nc, in_maps, core_ids=list(range(NCORES)))
    LAST_EXEC_S = (time.time() - t0,)
    LAST_HW_NS = None

    y = np.empty((NB, 8), np.float32)
    for c in range(NCORES):
        y[np.arange(NLB) * NCORES + c] = res.results[c]["yT"].T
    return y


# revision 16
# speedup vs baseline: 123.2521x; 1.3705x over previous
"""HGNN (2-layer hetero GraphSAGE + 8 heads) on 8 trn2 NeuronCores.

Single fused SPMD launch. Nodes are dst-interleaved (core = v % 8,
local = v // 8); each core receives only its node shard (bf16) plus edge
window metadata. On device:

  1. AllGather the x_b / x_s shards into full gather tables (core-block
     row order; gather indices are pre-translated on host).
  2. Layer 1: per 512-dst-column PSUM group, 128-edge windows (sorted by
     dst, cut on a column grid uniform across cores) are gathered with
     indirect DMA; a 0/1 selection matrix sel[e, j] = (rel[e] == j) built
     by one DVE is_equal feeds PE accumulation g.T @ sel -> raw sums s^T;
     the scatter-mean divides by per-dst counts via a DMA-broadcast
     1/cnt row. Dense stage + leaky-relu as usual; outputs are PE-
     transposed back to node-major and AllGathered into layer-2 tables.
  3. Layer 2 reuses the *same* window metadata (same edges, same table
     layout) against the layer-1 tables, then applies the 8-head
     classifier -> yT [8, NLB] per core (the only launch output).

kernel() runs one warm-up launch (hits the persistent jax compilation
cache) and then one timed launch; LAST_EXEC_S reports the timed
device-launch wall.
"""
import os
import time
import numpy as np

import jax
jax.config.update("jax_compilation_cache_dir",
                  os.path.expanduser("~/.cache/hgnn_jaxcache"))
jax.config.update("jax_persistent_cache_min_entry_size_bytes", -1)
jax.config.update("jax_persistent_cache_min_compile_time_secs", 0.0)

import ml_dtypes
import concourse.bass as bass
import concourse.bacc as bacc
import concourse.mybir as mybir
import concourse.tile as tile
from concourse.bass_utils import run_bass_kernel_spmd

P = 128
D = 128
NCORES = 8
GROUP = 512       # psum columns per accumulation group
S = 128           # max dst-column span per 128-edge window
BUCK = 25000      # src table rows per int16 gather bucket
NB, NS = 100000, 50000
NLB, NLS = NB // NCORES, NS // NCORES   # 12500, 6250
BF16 = ml_dtypes.bfloat16
QS = 26.0         # int8 feature quantization: code = rint(x * QS)


# ---------------------------------------------------------------- host prep
def _tr(v, nl):
    """Global node id -> row in the core-block AllGather table."""
    return (v % NCORES) * nl + v // NCORES


def _prep_type(src_t, dst, n_tab, n_loc):
    """Shard edges by dst core and pack 128-edge windows on a column grid
    uniform across cores (min-over-cores advance), bucketed by src table
    row so gather indices fit int16.

    src_t: edge source *table rows* (already translated), dst: global dst.
    Returns (idx16: per bucket [NCORES, 16, cols] int16,
             rel   [NCORES, P, Wtot] int8 (-1 pad),
             invc  [NCORES, n_loc] f32,
             groups: per group list of (bucket, k_local, col_off, span),
             gb_meta: per group dict bucket -> (idx slot base, Nk))."""
    nbuck = n_tab // BUCK
    ngroups = -(-n_loc // GROUP)
    core = dst % NCORES
    loc = dst // NCORES
    pcb = [[None] * nbuck for _ in range(NCORES)]
    cumb = [[None] * nbuck for _ in range(NCORES)]
    invc = np.empty((NCORES, n_loc), np.float32)
    for cc in range(NCORES):
        m = core == cc
        s, d = src_t[m], loc[m]
        o = np.argsort(d, kind="stable")
        s, d = s[o], d[o]
        invc[cc] = 1.0 / np.maximum(np.bincount(d, minlength=n_loc), 1)
        for b in range(nbuck):
            mb = (s >= b * BUCK) & (s < (b + 1) * BUCK)
            pcb[cc][b] = (s[mb] - b * BUCK, d[mb])
            cntb = np.bincount(d[mb], minlength=n_loc)
            cumb[cc][b] = np.concatenate([[0], np.cumsum(cntb)])

    groups, gb_meta = [], []
    rel_cols = [[] for _ in range(NCORES)]
    idx_flat = [[[] for _ in range(nbuck)] for _ in range(NCORES)]
    idx_base = [0] * nbuck
    for g in range(ngroups):
        c0, c1 = g * GROUP, min((g + 1) * GROUP, n_loc)
        wins, meta = [], {}
        for b in range(nbuck):
            k_local = 0
            c = c0
            while c < c1:
                span = min(S, c1 - c)
                while span > 1:
                    ok = all(cumb[cc][b][c + span] - cumb[cc][b][c] <= P
                             for cc in range(NCORES))
                    if ok:
                        break
                    span -= 1
                for cc in range(NCORES):
                    s_arr, d_arr = pcb[cc][b]
                    a2, b2 = cumb[cc][b][c], cumb[cc][b][c + span]
                    n = b2 - a2
                    assert n <= P
                    icol = np.zeros(P, np.int16)
                    rcol = np.full(P, -1, np.int8)
                    icol[:n] = s_arr[a2:b2].astype(np.int16)
                    rcol[:n] = (d_arr[a2:b2] - c).astype(np.int8)
                    idx_flat[cc][b].append(icol)
                    rel_cols[cc].append(rcol)
                wins.append((b, k_local, c - c0, span))
                k_local += 1
                c += span
            if k_local:
                meta[b] = (idx_base[b], k_local * P)
                idx_base[b] += k_local * P
        groups.append(wins)
        gb_meta.append(meta)

    idx16 = []
    for b in range(nbuck):
        per_core = []
        for cc in range(NCORES):
            flat = (np.concatenate(idx_flat[cc][b]) if idx_flat[cc][b]
                    else np.zeros(256, np.int16))
            per_core.append(np.ascontiguousarray(flat.reshape(-1, 16).T))
        idx16.append(np.stack(per_core))                 # [NCORES, 16, cols]
    rel = np.stack([np.stack(cs, 1) for cs in rel_cols]).astype(np.int8)
    return idx16, rel, invc, groups, gb_meta


# ------------------------------------------------------------- device build
def _build(types):
    """types: dict name -> dict(bcols, Wtot, groups, gb_meta, tab ('b'|'s'))."""
    nc = bacc.Bacc("TRN2", target_bir_lowering=False, debug=False,
                   num_devices=NCORES)
    f32, bf16 = mybir.dt.float32, mybir.dt.bfloat16
    i16, i8, i32 = mybir.dt.int16, mybir.dt.int8, mybir.dt.int32

    d_xb = nc.dram_tensor("xb", [NLB, P], i8, kind="ExternalInput")
    d_xs = nc.dram_tensor("xs", [NLS, P], i8, kind="ExternalInput")
    # 8 stacked [D, D] bf16 mats + WhT [D, 8]:
    # Wlbb0 Wlsb0 Wrb0 Wlbs0 Wrs0 Wlbb1 Wlsb1 Wrb1 WhT
    # (Wrb0 / Wrs0 are pre-scaled by 1/QS on the host: the layer-1 x-term
    # is computed on int8 codes)
    d_wb = nc.dram_tensor("wb", [P, 8 * D + 8], bf16, kind="ExternalInput")
    # bias columns f32: bb0 bs0 bb1 bh(first 8 rows)
    d_bias = nc.dram_tensor("bias", [P, 4], f32, kind="ExternalInput")
    # 1/cnt rows; layer-1 rows are pre-multiplied by 1/QS (codes -> values)
    d_iv = {"bb0": nc.dram_tensor("ivb", [1, NLB], f32, kind="ExternalInput"),
            "sb0": nc.dram_tensor("ivs", [1, NLB], f32, kind="ExternalInput"),
            "bs0": nc.dram_tensor("ivq", [1, NLS], f32, kind="ExternalInput"),
            "bb1": nc.dram_tensor("jvb", [1, NLB], f32, kind="ExternalInput"),
            "sb1": nc.dram_tensor("jvs", [1, NLB], f32, kind="ExternalInput")}
    d_idx = {t: [nc.dram_tensor(f"idx_{t}_{b}", [16, c], i16,
                                kind="ExternalInput")
                 for b, c in enumerate(types[t]["bcols"])] for t in types}
    d_rel = {t: nc.dram_tensor(f"rel_{t}", [P, types[t]["Wtot"]], i8,
                               kind="ExternalInput") for t in types}
    d_yT = nc.dram_tensor("yT", [8, NLB], f32, kind="ExternalOutput")

    from contextlib import ExitStack
    with tile.TileContext(nc) as tc, ExitStack() as ctx:
        wpool = ctx.enter_context(tc.tile_pool(name="w", bufs=1))
        dpool = ctx.enter_context(tc.tile_pool(name="dr", bufs=1, space="DRAM"))
        gpool = ctx.enter_context(tc.tile_pool(name="g", bufs=6))
        selpool = ctx.enter_context(tc.tile_pool(name="sel", bufs=2))
        mpool = ctx.enter_context(tc.tile_pool(name="m", bufs=2))
        spool = ctx.enter_context(tc.tile_pool(name="s", bufs=3))
        appool = ctx.enter_context(tc.tile_pool(name="ap", bufs=3, space="PSUM"))
        s2pool = ctx.enter_context(tc.tile_pool(name="s2", bufs=2, space="PSUM"))
        trpool = ctx.enter_context(tc.tile_pool(name="tr", bufs=2, space="PSUM"))
        hpool = ctx.enter_context(tc.tile_pool(name="h", bufs=1, space="PSUM"))

        # ---- DRAM scratch: bounce shards, gather tables, layer-1 staging
        bounce_b = dpool.tile([NLB, P], i8, tag="bnb")
        bounce_s = dpool.tile([NLS, P], i8, tag="bns")
        tab8_b = dpool.tile([NB, P], i8, tag="t8b")
        tab8_s = dpool.tile([NS, P], i8, tag="t8s")
        tab_b0 = dpool.tile([NB, P], bf16, tag="tb0")
        tab_s0 = dpool.tile([NS, P], bf16, tag="ts0")
        tab_b1 = dpool.tile([NB, P], bf16, tag="tb1")
        tab_s1 = dpool.tile([NS, P], bf16, tag="ts1")
        nb_nm = dpool.tile([NLB, P], bf16, tag="nbm")   # L1 b out, node-major
        ns_nm = dpool.tile([NLS, P], bf16, tag="nsm")
        d_nbT = dpool.tile([P, NLB], bf16, tag="nbt")   # L1 b out, feat-major

        grp = [list(range(NCORES))]
        nc.sync.dma_start(bounce_b[:], d_xb[:])
        nc.gpsimd.collective_compute(
            "AllGather", mybir.AluOpType.bypass, replica_groups=grp,
            ins=[bounce_b[:].opt()], outs=[tab8_b[:].opt()])
        nc.sync.dma_start(bounce_s[:], d_xs[:])
        nc.gpsimd.collective_compute(
            "AllGather", mybir.AluOpType.bypass, replica_groups=grp,
            ins=[bounce_s[:].opt()], outs=[tab8_s[:].opt()])

        # widen the int8 code tables to bf16 (codes are exact in bf16) so
        # dma_gather sees 256-byte rows
        def cast_range(tab8, tabf, j0, rows):
            if rows >= P:
                b = rows // P
                t8 = gpool.tile([P, b * P], i8, tag="c8")
                tf = gpool.tile([P, b * P], bf16, tag="cf")
                nc.sync.dma_start(
                    t8[:], tab8[j0:j0 + rows, :]
                    .rearrange("(a b) d -> a (b d)", a=P))
                nc.vector.tensor_copy(out=tf[:], in_=t8[:])
                nc.sync.dma_start(
                    tabf[j0:j0 + rows, :]
                    .rearrange("(a b) d -> a (b d)", a=P), tf[:])
            else:
                t8 = gpool.tile([P, P], i8, tag="c8")
                tf = gpool.tile([P, P], bf16, tag="cf")
                nc.sync.dma_start(t8[:rows, :], tab8[j0:j0 + rows, :])
                nc.vector.tensor_copy(out=tf[:rows, :], in_=t8[:rows, :])
                nc.sync.dma_start(tabf[j0:j0 + rows, :], tf[:rows, :])

        for tab8, tabf, n in ((tab8_b, tab_b0, NB), (tab8_s, tab_s0, NS)):
            j0 = 0
            while j0 < n:
                rows = min(16 * P, ((n - j0) // P) * P) or (n - j0)
                cast_range(tab8, tabf, j0, rows)
                j0 += rows

        # ---- constants: weights, iota row, identity
        t_w = wpool.tile([P, 8 * D + 8], bf16, tag="wb")
        nc.sync.dma_start(t_w[:], d_wb[:])
        wm = {n: t_w[:, i * D:(i + 1) * D] for i, n in enumerate(
            ["Wlbb0", "Wlsb0", "Wrb0", "Wlbs0", "Wrs0",
             "Wlbb1", "Wlsb1", "Wrb1"])}
        w_WhT = t_w[:, 8 * D:8 * D + 8]
        t_bias = wpool.tile([P, 4], f32, tag="bias")
        nc.sync.dma_start(t_bias[:], d_bias[:])
        b_bb0, b_bs0, b_bb1 = (t_bias[:, i:i + 1] for i in range(3))
        b_h = t_bias[:8, 3:4]

        t_ii = wpool.tile([P, S], i32, tag="ii")
        nc.gpsimd.iota(t_ii[:], pattern=[[1, S]], base=0, channel_multiplier=0)
        t_iota = wpool.tile([P, S], f32, tag="iota")
        nc.vector.tensor_copy(out=t_iota[:], in_=t_ii[:])
        t_ip = wpool.tile([P, 1], i32, tag="ip")
        nc.gpsimd.iota(t_ip[:], pattern=[[0, 1]], base=0, channel_multiplier=1)
        t_ipf = wpool.tile([P, 1], f32, tag="ipf")
        nc.vector.tensor_copy(out=t_ipf[:], in_=t_ip[:])
        t_id = wpool.tile([P, P], bf16, tag="ident")
        nc.vector.tensor_tensor(out=t_id[:], in0=t_iota[:],
                                in1=t_ipf[:].to_broadcast([P, P]),
                                op=mybir.AluOpType.is_equal)

        # ---- resident per-type idx (replicated 16->128 on device) and rel f32
        t_idx, t_rel = {}, {}
        for t in types:
            t_idx[t] = []
            for b, cols in enumerate(types[t]["bcols"]):
                ti = wpool.tile([P, cols], i16, tag=f"ix_{t}_{b}")
                for k in range(8):
                    nc.sync.dma_start(ti[16 * k:16 * (k + 1), :], d_idx[t][b][:])
                t_idx[t].append(ti)
            tr8 = wpool.tile([P, types[t]["Wtot"]], i8, tag=f"r8_{t}")
            nc.sync.dma_start(tr8[:], d_rel[t][:])
            trf = wpool.tile([P, types[t]["Wtot"]], f32, tag=f"rf_{t}")
            nc.vector.tensor_copy(out=trf[:], in_=tr8[:])
            t_rel[t] = trf

        def aggregate(tname, g, wbase, tab, tab_i8):
            """Accumulate one group's scatter-sum into PSUM: returns
            (psum tile [P, GROUP] f32, ncols)."""
            ty = types[tname]
            wins = ty["groups"][g]
            meta = ty["gb_meta"][g]
            Wg = len(wins)
            ncols = max(c + sp for (_, _, c, sp) in wins)
            t_sel = selpool.tile([P, Wg * S], bf16, tag="sel")
            sel3 = t_sel[:].rearrange("p (w s) -> p w s", w=Wg)
            rel = t_rel[tname]
            nc.vector.tensor_tensor(
                out=sel3,
                in0=rel[:, wbase:wbase + Wg, None].to_broadcast([P, Wg, S]),
                in1=t_iota[:, None, :].to_broadcast([P, Wg, S]),
                op=mybir.AluOpType.is_equal)
            gtiles = {}
            for b, (sbase, Nk) in sorted(meta.items()):
                t_gb = gpool.tile([P, (Nk // P) * D], bf16, tag="gb")
                nc.gpsimd.dma_gather(
                    out_ap=t_gb[:].rearrange("p (k d) -> p k d", k=Nk // P),
                    in_ap=tab[b * BUCK:(b + 1) * BUCK, :],
                    idxs_ap=t_idx[tname][b][:, sbase // 16:(sbase + Nk) // 16],
                    num_idxs=Nk, num_idxs_reg=Nk, elem_size=D,
                    single_packet=False)
                gtiles[b] = t_gb
            t_ps = appool.tile([P, GROUP], mybir.dt.float32, space="PSUM",
                               tag="agg")
            for w, (b, k, coff, span) in enumerate(wins):
                nc.tensor.matmul(
                    t_ps[:, coff:coff + span],
                    lhsT=gtiles[b][:, k * D:(k + 1) * D],
                    rhs=t_sel[:, w * S:w * S + span],
                    start=(w == 0), stop=(w == Wg - 1))
            return t_ps, ncols

        def scale_mean(ivkey, g, t_ps, ncols):
            """m^T = s^T * (1/cnt) broadcast across partitions -> bf16."""
            t_iv = spool.tile([P, GROUP], mybir.dt.float32, tag="iv")
            nc.sync.dma_start(
                t_iv[:, :ncols],
                d_iv[ivkey][0:1, g * GROUP:g * GROUP + ncols]
                .to_broadcast([P, ncols]))
            t_m = mpool.tile([P, GROUP], mybir.dt.bfloat16, tag=f"m_{ivkey}")
            nc.vector.tensor_tensor(out=t_m[:, :ncols], in0=t_ps[:, :ncols],
                                    in1=t_iv[:, :ncols],
                                    op=mybir.AluOpType.mult)
            return t_m

        def xT_blocks(src_dram, g, ncols, n_loc):
            """Load node-major int8 rows for this group and PE-transpose
            into a feature-major [P, ncols] bf16 code tile."""
            t_x = spool.tile([P, GROUP], mybir.dt.bfloat16, tag="xg")
            j0 = 0
            while j0 < ncols:
                w = min(P, ncols - j0)
                t_b8 = gpool.tile([P, P], i8, tag="xblk8")
                nc.sync.dma_start(
                    t_b8[:w, :],
                    src_dram[g * GROUP + j0:g * GROUP + j0 + w, :])
                t_blk = gpool.tile([P, P], mybir.dt.bfloat16, tag="xblk")
                nc.vector.tensor_copy(out=t_blk[:w, :], in_=t_b8[:w, :])
                ps_t = trpool.tile([P, P], mybir.dt.bfloat16, space="PSUM",
                                   tag="tr")
                nc.tensor.transpose(ps_t[:, :w], t_blk[:w, :], t_id[:w, :w])
                nc.vector.tensor_copy(out=t_x[:, j0:j0 + w], in_=ps_t[:, :w])
                j0 += w
            return t_x

        def emit_node_major(t_o, dst_dram, g, ncols):
            """PE-transpose feature-major output back to node-major rows."""
            j0 = 0
            while j0 < ncols:
                w = min(P, ncols - j0)
                ps_t = trpool.tile([P, P], mybir.dt.bfloat16, space="PSUM",
                                   tag="tr")
                nc.tensor.transpose(ps_t[:w, :], t_o[:, j0:j0 + w], t_id[:])
                t_nm = gpool.tile([P, P], mybir.dt.bfloat16, tag="nm")
                nc.vector.tensor_copy(out=t_nm[:w, :], in_=ps_t[:w, :])
                nc.sync.dma_start(
                    dst_dram[g * GROUP + j0:g * GROUP + j0 + w, :],
                    t_nm[:w, :])
                j0 += w

        # ---------------- layer 1, s-dst groups (first: frees tab_s1 early)
        wb_bs = 0
        for g in range(len(types["bs"]["groups"])):
            ps_agg, ncols = aggregate("bs", g, wb_bs, tab_b0, True)
            wb_bs += len(types["bs"]["groups"][g])
            t_m = scale_mean("bs0", g, ps_agg, ncols)
            t_x = xT_blocks(d_xs, g, ncols, NLS)
            ps2 = s2pool.tile([P, GROUP], mybir.dt.float32, space="PSUM",
                              tag="s2")
            nc.tensor.matmul(ps2[:, :ncols], lhsT=wm["Wlbs0"],
                             rhs=t_m[:, :ncols], start=True, stop=False)
            nc.tensor.matmul(ps2[:, :ncols], lhsT=wm["Wrs0"],
                             rhs=t_x[:, :ncols], start=False, stop=True)
            t_o = spool.tile([P, GROUP], mybir.dt.bfloat16, tag="ob")
            nc.scalar.activation(out=t_o[:, :ncols], in_=ps2[:, :ncols],
                                 func=mybir.ActivationFunctionType.Lrelu,
                                 bias=b_bs0, alpha=0.01)
            emit_node_major(t_o, ns_nm, g, ncols)
        nc.gpsimd.collective_compute(
            "AllGather", mybir.AluOpType.bypass, replica_groups=grp,
            ins=[ns_nm[:].opt()], outs=[tab_s1[:].opt()])

        # ---------------- layer 1, b-dst groups
        wb_bb = 0
        wb_sb = 0
        for g in range(len(types["bb"]["groups"])):
            ps_bb, ncols = aggregate("bb", g, wb_bb, tab_b0, True)
            wb_bb += len(types["bb"]["groups"][g])
            m_bb = scale_mean("bb0", g, ps_bb, ncols)
            has_sb = bool(types["sb"]["groups"][g])
            if has_sb:
                ps_sb, ncols_sb = aggregate("sb", g, wb_sb, tab_s0, True)
                wb_sb += len(types["sb"]["groups"][g])
                m_sb = scale_mean("sb0", g, ps_sb, ncols_sb)
            t_x = xT_blocks(d_xb, g, ncols, NLB)
            ps2 = s2pool.tile([P, GROUP], mybir.dt.float32, space="PSUM",
                              tag="s2")
            nc.tensor.matmul(ps2[:, :ncols], lhsT=wm["Wlbb0"],
                             rhs=m_bb[:, :ncols], start=True, stop=False)
            if has_sb:
                nc.tensor.matmul(ps2[:, :ncols_sb], lhsT=wm["Wlsb0"],
                                 rhs=m_sb[:, :ncols_sb], start=False,
                                 stop=False)
            nc.tensor.matmul(ps2[:, :ncols], lhsT=wm["Wrb0"],
                             rhs=t_x[:, :ncols], start=False, stop=True)
            t_o = spool.tile([P, GROUP], mybir.dt.bfloat16, tag="ob")
            nc.scalar.activation(out=t_o[:, :ncols], in_=ps2[:, :ncols],
                                 func=mybir.ActivationFunctionType.Lrelu,
                                 bias=b_bb0, alpha=0.01)
            nc.sync.dma_start(d_nbT[:, g * GROUP:g * GROUP + ncols],
                              t_o[:, :ncols])
            emit_node_major(t_o, nb_nm, g, ncols)
        nc.gpsimd.collective_compute(
            "AllGather", mybir.AluOpType.bypass, replica_groups=grp,
            ins=[nb_nm[:].opt()], outs=[tab_b1[:].opt()])

        # ---------------- layer 2, b-dst groups (+ heads)
        wb_bb = 0
        wb_sb = 0
        for g in range(len(types["bb"]["groups"])):
            ps_bb, ncols = aggregate("bb", g, wb_bb, tab_b1, False)
            wb_bb += len(types["bb"]["groups"][g])
            m_bb = scale_mean("bb1", g, ps_bb, ncols)
            has_sb = bool(types["sb"]["groups"][g])
            if has_sb:
                ps_sb, ncols_sb = aggregate("sb", g, wb_sb, tab_s1, False)
                wb_sb += len(types["sb"]["groups"][g])
                m_sb = scale_mean("sb1", g, ps_sb, ncols_sb)
            t_x = spool.tile([P, GROUP], mybir.dt.bfloat16, tag="xg")
            nc.sync.dma_start(t_x[:, :ncols],
                              d_nbT[:, g * GROUP:g * GROUP + ncols])
            ps2 = s2pool.tile([P, GROUP], mybir.dt.float32, space="PSUM",
                              tag="s2")
            nc.tensor.matmul(ps2[:, :ncols], lhsT=wm["Wlbb1"],
                             rhs=m_bb[:, :ncols], start=True, stop=False)
            if has_sb:
                nc.tensor.matmul(ps2[:, :ncols_sb], lhsT=wm["Wlsb1"],
                                 rhs=m_sb[:, :ncols_sb], start=False,
                                 stop=False)
            nc.tensor.matmul(ps2[:, :ncols], lhsT=wm["Wrb1"],
                             rhs=t_x[:, :ncols], start=False, stop=True)
            t_o = spool.tile([P, GROUP], mybir.dt.bfloat16, tag="ob")
            nc.scalar.activation(out=t_o[:, :ncols], in_=ps2[:, :ncols],
                                 func=mybir.ActivationFunctionType.Lrelu,
                                 bias=b_bb1, alpha=0.01)
            ps3 = hpool.tile([8, GROUP], mybir.dt.float32, space="PSUM",
                             tag="hd")
            nc.tensor.matmul(ps3[:, :ncols], lhsT=w_WhT, rhs=t_o[:, :ncols],
                             start=True, stop=True)
            t_y = spool.tile([8, GROUP], mybir.dt.float32, tag="yt")
            nc.vector.tensor_scalar_add(t_y[:, :ncols], ps3[:, :ncols], b_h)
            nc.sync.dma_start(d_yT[:, g * GROUP:g * GROUP + ncols],
                              t_y[:, :ncols])

    nc.compile()
    return nc


LAST_HW_NS = None
LAST_EXEC_S = None
LAST_WARM_S = None


def kernel(x_b, x_s, Wl, bl, Wr, Wh, bh, ei_bb, ei_sb, ei_bs):
    x_b = np.asarray(x_b, np.float32)
    x_s = np.asarray(x_s, np.float32)
    Wl = np.asarray(Wl, np.float32)
    bl = np.asarray(bl, np.float32)
    Wr = np.asarray(Wr, np.float32)
    Wh = np.asarray(Wh, np.float32)
    bh = np.asarray(bh, np.float32)
    ei_bb = np.asarray(ei_bb).astype(np.int64)
    ei_sb = np.asarray(ei_sb).astype(np.int64)
    ei_bs = np.asarray(ei_bs).astype(np.int64)

    # window packing (indices pre-translated into AllGather table rows;
    # identical metadata serves both layers)
    i_bb, r_bb, v_bb, g_bb, m_bb = _prep_type(
        _tr(ei_bb[0], NLB), ei_bb[1], NB, NLB)
    i_sb, r_sb, v_sb, g_sb, m_sb = _prep_type(
        _tr(ei_sb[0], NLS), ei_sb[1], NS, NLB)
    i_bs, r_bs, v_bs, g_bs, m_bs = _prep_type(
        _tr(ei_bs[0], NLB), ei_bs[1], NB, NLS)

    types = {
        "bb": {"bcols": [a.shape[2] for a in i_bb], "Wtot": r_bb.shape[2],
               "groups": g_bb, "gb_meta": m_bb},
        "sb": {"bcols": [a.shape[2] for a in i_sb], "Wtot": r_sb.shape[2],
               "groups": g_sb, "gb_meta": m_sb},
        "bs": {"bcols": [a.shape[2] for a in i_bs], "Wtot": r_bs.shape[2],
               "groups": g_bs, "gb_meta": m_bs},
    }
    nc = _build(types)

    # weight payload (bf16) + bias columns (f32); layer-1 Wr pre-scaled by
    # 1/QS (applied to int8 codes)
    wmats = [Wl[0, 0], Wl[0, 1], (Wr[0, 0] + Wr[0, 1]) / QS,
             Wl[0, 2], Wr[0, 2] / QS,
             Wl[1, 0], Wl[1, 1], Wr[1, 0] + Wr[1, 1]]
    wb_np = np.zeros((P, 8 * D + 8), BF16)
    for i, M in enumerate(wmats):
        wb_np[:, i * D:(i + 1) * D] = M.astype(BF16)
    wb_np[:, 8 * D:8 * D + 8] = Wh.T.astype(BF16)
    bias_np = np.zeros((P, 4), np.float32)
    bias_np[:, 0] = bl[0, 0] + bl[0, 1]
    bias_np[:, 1] = bl[0, 2]
    bias_np[:, 2] = bl[1, 0] + bl[1, 1]
    bias_np[:8, 3] = bh

    def q8(a):
        return np.clip(np.rint(a * QS), -127, 127).astype(np.int8)

    in_maps = []
    for c in range(NCORES):
        im = {
            "xb": q8(np.ascontiguousarray(x_b[c::NCORES])),
            "xs": q8(np.ascontiguousarray(x_s[c::NCORES])),
            "wb": wb_np, "bias": bias_np,
            "ivb": v_bb[c][None, :] / np.float32(QS),
            "ivs": v_sb[c][None, :] / np.float32(QS),
            "ivq": v_bs[c][None, :] / np.float32(QS),
            "jvb": v_bb[c][None, :], "jvs": v_sb[c][None, :],
            "rel_bb": r_bb[c], "rel_sb": r_sb[c], "rel_bs": r_bs[c],
        }
        for t, arrs in (("bb", i_bb), ("sb", i_sb), ("bs", i_bs)):
            for b, a in enumerate(arrs):
                im[f"idx_{t}_{b}"] = a[c]
        in_maps.append(im)

    global LAST_HW_NS, LAST_EXEC_S, LAST_WARM_S
    t0 = time.time()
    run_bass_kernel_spmd(nc, in_maps, core_ids=list(range(NCORES)))
    LAST_WARM_S = time.time() - t0

    t0 = time.time()
    res = run_bass_kernel_spmd(nc, in_maps, core_ids=list(range(NCORES)))
    LAST_EXEC_S = (time.time() - t0,)
    LAST_HW_NS = None

    y = np.empty((NB, 8), np.float32)
    for c in range(NCORES):
        y[np.arange(NLB) * NCORES + c] = res.results[c]["yT"].T
    return y


# revision 17
# speedup vs baseline: 134.4920x; 1.0912x over previous
"""HGNN (2-layer hetero GraphSAGE + 8 heads) on 8 trn2 NeuronCores.

Single fused SPMD launch. Nodes are dst-interleaved (core = v % 8,
local = v // 8); each core receives only its node shard (int8 codes,
x ~= code / QS) plus edge window metadata, packed into 6 input arrays
to minimize per-array PJRT overhead. On device:

  1. AllGather the int8 shards into full code tables (core-block row
     order; gather indices pre-translated on host), then widen to bf16
     (codes are exact in bf16; dma_gather wants 256B rows).
  2. Layer 1: per 512-dst-column PSUM group, 128-edge windows (dst-
     sorted, cut on a column grid uniform across all cores so one
     program serves SPMD) are gathered by indirect DMA; a 0/1 selection
     matrix sel[e, j] = (rel[e] == j) from one DVE is_equal feeds PE
     accumulation g.T @ sel -> raw sums s^T; scatter-mean multiplies by
     a DMA-broadcast (1/cnt)/QS row. Dense stage (layer-1 Wr pre-scaled
     by 1/QS) + bias + leaky-relu; outputs are PE-transposed to node-
     major and AllGathered into the layer-2 bf16 tables.
  3. Layer 2 reuses the *same* window metadata against the layer-1
     tables (Wl pre-scaled by QS to reuse the layer-1 1/cnt rows), then
     the 8-head classifier -> yT [8, NLB] f32 per core (only output).

kernel() runs one warm-up launch (hits the persistent jax compilation
cache) then one timed launch; LAST_EXEC_S is the timed launch wall.
"""
import os
import time
import numpy as np

import jax
jax.config.update("jax_compilation_cache_dir",
                  os.path.expanduser("~/.cache/hgnn_jaxcache"))
jax.config.update("jax_persistent_cache_min_entry_size_bytes", -1)
jax.config.update("jax_persistent_cache_min_compile_time_secs", 0.0)

import ml_dtypes
import concourse.bass as bass
import concourse.bacc as bacc
import concourse.mybir as mybir
import concourse.tile as tile
from concourse.bass_utils import run_bass_kernel_spmd

P = 128
D = 128
NCORES = 8
GROUP = 512       # psum columns per accumulation group
S = 128           # max dst-column span per 128-edge window
BUCK = 25000      # src table rows per int16 gather bucket
NB, NS = 100000, 50000
NLB, NLS = NB // NCORES, NS // NCORES   # 12500, 6250
BF16 = ml_dtypes.bfloat16
QS = 26.0         # int8 feature quantization: code = rint(x * QS)
TYPES = ("bb", "sb", "bs")


# ---------------------------------------------------------------- host prep
def _tr(v, nl):
    """Global node id -> row in the core-block AllGather table."""
    return (v % NCORES) * nl + v // NCORES


def _prep_type(src_t, dst, n_tab, n_loc):
    """Shard edges by dst core and pack 128-edge windows on a column grid
    uniform across cores (min-over-cores advance), bucketed by src table
    row so gather indices fit int16.

    src_t: edge source *table rows* (already translated), dst: global dst.
    Returns (idx16: per bucket [NCORES, 16, cols] int16,
             rel   [NCORES, P, Wtot] int8 (-1 pad),
             invc  [NCORES, n_loc] f32,
             groups: per group list of (bucket, k_local, col_off, span),
             gb_meta: per group dict bucket -> (idx slot base, Nk))."""
    nbuck = n_tab // BUCK
    ngroups = -(-n_loc // GROUP)
    core = dst % NCORES
    loc = dst // NCORES
    pcb = [[None] * nbuck for _ in range(NCORES)]
    cumb = [[None] * nbuck for _ in range(NCORES)]
    invc = np.empty((NCORES, n_loc), np.float32)
    for cc in range(NCORES):
        m = core == cc
        s, d = src_t[m], loc[m]
        o = np.argsort(d, kind="stable")
        s, d = s[o], d[o]
        invc[cc] = 1.0 / np.maximum(np.bincount(d, minlength=n_loc), 1)
        for b in range(nbuck):
            mb = (s >= b * BUCK) & (s < (b + 1) * BUCK)
            pcb[cc][b] = (s[mb] - b * BUCK, d[mb])
            cntb = np.bincount(d[mb], minlength=n_loc)
            cumb[cc][b] = np.concatenate([[0], np.cumsum(cntb)])

    groups, gb_meta = [], []
    rel_cols = [[] for _ in range(NCORES)]
    idx_flat = [[[] for _ in range(nbuck)] for _ in range(NCORES)]
    idx_base = [0] * nbuck
    for g in range(ngroups):
        c0, c1 = g * GROUP, min((g + 1) * GROUP, n_loc)
        wins, meta = [], {}
        for b in range(nbuck):
            k_local = 0
            c = c0
            while c < c1:
                span = min(S, c1 - c)
                while span > 1:
                    ok = all(cumb[cc][b][c + span] - cumb[cc][b][c] <= P
                             for cc in range(NCORES))
                    if ok:
                        break
                    span -= 1
                for cc in range(NCORES):
                    s_arr, d_arr = pcb[cc][b]
                    a2, b2 = cumb[cc][b][c], cumb[cc][b][c + span]
                    n = b2 - a2
                    assert n <= P
                    icol = np.zeros(P, np.int16)
                    rcol = np.full(P, -1, np.int8)
                    icol[:n] = s_arr[a2:b2].astype(np.int16)
                    rcol[:n] = (d_arr[a2:b2] - c).astype(np.int8)
                    idx_flat[cc][b].append(icol)
                    rel_cols[cc].append(rcol)
                wins.append((b, k_local, c - c0, span))
                k_local += 1
                c += span
            if k_local:
                meta[b] = (idx_base[b], k_local * P)
                idx_base[b] += k_local * P
        groups.append(wins)
        gb_meta.append(meta)

    idx16 = []
    for b in range(nbuck):
        per_core = []
        for cc in range(NCORES):
            flat = (np.concatenate(idx_flat[cc][b]) if idx_flat[cc][b]
                    else np.zeros(256, np.int16))
            per_core.append(np.ascontiguousarray(flat.reshape(-1, 16).T))
        idx16.append(np.stack(per_core))                 # [NCORES, 16, cols]
    rel = np.stack([np.stack(cs, 1) for cs in rel_cols]).astype(np.int8)
    return idx16, rel, invc, groups, gb_meta


# ------------------------------------------------------------- device build
def _build(types, totc, totw):
    """types: name -> dict(bcols, ioff (per-bucket col offset into the idx
    blob), roff (col offset into the rel blob), Wtot, groups, gb_meta)."""
    nc = bacc.Bacc("TRN2", target_bir_lowering=False, debug=False,
                   num_devices=NCORES)
    f32, bf16 = mybir.dt.float32, mybir.dt.bfloat16
    i16, i8, i32 = mybir.dt.int16, mybir.dt.int8, mybir.dt.int32

    d_x8 = nc.dram_tensor("x8", [NLB + NLS, P], i8, kind="ExternalInput")
    # 8 stacked [D, D] bf16 mats + WhT [D, 8]:
    # Wlbb0 Wlsb0 Wrb0/QS Wlbs0 Wrs0/QS Wlbb1*QS Wlsb1*QS Wrb1 WhT
    d_wb = nc.dram_tensor("wb", [P, 8 * D + 8], bf16, kind="ExternalInput")
    # bias columns f32: bb0 bs0 bb1 bh(first 8 rows)
    d_bias = nc.dram_tensor("bias", [P, 4], f32, kind="ExternalInput")
    # (1/cnt)/QS rows: bb at 0, sb at NLB, bs at 2*NLB
    d_iv = nc.dram_tensor("iv", [1, 2 * NLB + NLS], f32, kind="ExternalInput")
    d_idx = nc.dram_tensor("idx", [16, totc], i16, kind="ExternalInput")
    d_rel = nc.dram_tensor("rel", [P, totw], i8, kind="ExternalInput")
    d_yT = nc.dram_tensor("yT", [8, NLB], f32, kind="ExternalOutput")
    IVOFF = {"bb": 0, "sb": NLB, "bs": 2 * NLB}

    from contextlib import ExitStack
    with tile.TileContext(nc) as tc, ExitStack() as ctx:
        wpool = ctx.enter_context(tc.tile_pool(name="w", bufs=1))
        dpool = ctx.enter_context(tc.tile_pool(name="dr", bufs=1, space="DRAM"))
        gpool = ctx.enter_context(tc.tile_pool(name="g", bufs=6))
        selpool = ctx.enter_context(tc.tile_pool(name="sel", bufs=2))
        mpool = ctx.enter_context(tc.tile_pool(name="m", bufs=2))
        spool = ctx.enter_context(tc.tile_pool(name="s", bufs=3))
        appool = ctx.enter_context(tc.tile_pool(name="ap", bufs=3, space="PSUM"))
        s2pool = ctx.enter_context(tc.tile_pool(name="s2", bufs=2, space="PSUM"))
        trpool = ctx.enter_context(tc.tile_pool(name="tr", bufs=2, space="PSUM"))
        hpool = ctx.enter_context(tc.tile_pool(name="h", bufs=1, space="PSUM"))

        # ---- DRAM scratch: bounce shards, gather tables, layer-1 staging
        bounce_b = dpool.tile([NLB, P], i8, tag="bnb")
        bounce_s = dpool.tile([NLS, P], i8, tag="bns")
        tab8_b = dpool.tile([NB, P], i8, tag="t8b")
        tab8_s = dpool.tile([NS, P], i8, tag="t8s")
        tab_b0 = dpool.tile([NB, P], bf16, tag="tb0")
        tab_s0 = dpool.tile([NS, P], bf16, tag="ts0")
        tab_b1 = dpool.tile([NB, P], bf16, tag="tb1")
        tab_s1 = dpool.tile([NS, P], bf16, tag="ts1")
        nb_nm = dpool.tile([NLB, P], bf16, tag="nbm")   # L1 b out, node-major
        ns_nm = dpool.tile([NLS, P], bf16, tag="nsm")
        d_nbT = dpool.tile([P, NLB], bf16, tag="nbt")   # L1 b out, feat-major

        grp = [list(range(NCORES))]
        nc.sync.dma_start(bounce_b[:], d_x8[:NLB, :])
        nc.gpsimd.collective_compute(
            "AllGather", mybir.AluOpType.bypass, replica_groups=grp,
            ins=[bounce_b[:].opt()], outs=[tab8_b[:].opt()])
        nc.sync.dma_start(bounce_s[:], d_x8[NLB:, :])
        nc.gpsimd.collective_compute(
            "AllGather", mybir.AluOpType.bypass, replica_groups=grp,
            ins=[bounce_s[:].opt()], outs=[tab8_s[:].opt()])

        # widen the int8 code tables to bf16 so dma_gather sees 256B rows
        def cast_range(tab8, tabf, j0, rows):
            if rows >= P:
                b = rows // P
                t8 = gpool.tile([P, b * P], i8, tag="c8")
                tf = gpool.tile([P, b * P], bf16, tag="cf")
                nc.sync.dma_start(
                    t8[:], tab8[j0:j0 + rows, :]
                    .rearrange("(a b) d -> a (b d)", a=P))
                nc.vector.tensor_copy(out=tf[:], in_=t8[:])
                nc.sync.dma_start(
                    tabf[j0:j0 + rows, :]
                    .rearrange("(a b) d -> a (b d)", a=P), tf[:])
            else:
                t8 = gpool.tile([P, P], i8, tag="c8")
                tf = gpool.tile([P, P], bf16, tag="cf")
                nc.sync.dma_start(t8[:rows, :], tab8[j0:j0 + rows, :])
                nc.vector.tensor_copy(out=tf[:rows, :], in_=t8[:rows, :])
                nc.sync.dma_start(tabf[j0:j0 + rows, :], tf[:rows, :])

        for tab8, tabf, n in ((tab8_b, tab_b0, NB), (tab8_s, tab_s0, NS)):
            j0 = 0
            while j0 < n:
                rows = min(16 * P, ((n - j0) // P) * P) or (n - j0)
                cast_range(tab8, tabf, j0, rows)
                j0 += rows

        # ---- constants: weights, iota row, identity
        t_w = wpool.tile([P, 8 * D + 8], bf16, tag="wb")
        nc.sync.dma_start(t_w[:], d_wb[:])
        wm = {n: t_w[:, i * D:(i + 1) * D] for i, n in enumerate(
            ["Wlbb0", "Wlsb0", "Wrb0", "Wlbs0", "Wrs0",
             "Wlbb1", "Wlsb1", "Wrb1"])}
        w_WhT = t_w[:, 8 * D:8 * D + 8]
        t_bias = wpool.tile([P, 4], f32, tag="bias")
        nc.sync.dma_start(t_bias[:], d_bias[:])
        b_bb0, b_bs0, b_bb1 = (t_bias[:, i:i + 1] for i in range(3))
        b_h = t_bias[:8, 3:4]

        t_ii = wpool.tile([P, S], i32, tag="ii")
        nc.gpsimd.iota(t_ii[:], pattern=[[1, S]], base=0, channel_multiplier=0)
        t_iota = wpool.tile([P, S], f32, tag="iota")
        nc.vector.tensor_copy(out=t_iota[:], in_=t_ii[:])
        t_ip = wpool.tile([P, 1], i32, tag="ip")
        nc.gpsimd.iota(t_ip[:], pattern=[[0, 1]], base=0, channel_multiplier=1)
        t_ipf = wpool.tile([P, 1], f32, tag="ipf")
        nc.vector.tensor_copy(out=t_ipf[:], in_=t_ip[:])
        t_id = wpool.tile([P, P], bf16, tag="ident")
        nc.vector.tensor_tensor(out=t_id[:], in0=t_iota[:],
                                in1=t_ipf[:].to_broadcast([P, P]),
                                op=mybir.AluOpType.is_equal)

        # ---- resident idx blob (replicated 16->128 on device) and rel f32
        t_idx = wpool.tile([P, totc], i16, tag="idxb")
        for k in range(8):
            nc.sync.dma_start(t_idx[16 * k:16 * (k + 1), :], d_idx[:])
        t_r8 = wpool.tile([P, totw], i8, tag="rel8")
        nc.sync.dma_start(t_r8[:], d_rel[:])
        t_rel = wpool.tile([P, totw], f32, tag="relf")
        nc.vector.tensor_copy(out=t_rel[:], in_=t_r8[:])

        def aggregate(tname, g, wbase, tab):
            """Accumulate one group's scatter-sum into PSUM: returns
            (psum tile [P, GROUP] f32, ncols)."""
            ty = types[tname]
            wins = ty["groups"][g]
            meta = ty["gb_meta"][g]
            Wg = len(wins)
            ncols = max(c + sp for (_, _, c, sp) in wins)
            t_sel = selpool.tile([P, Wg * S], bf16, tag="sel")
            sel3 = t_sel[:].rearrange("p (w s) -> p w s", w=Wg)
            r0 = ty["roff"] + wbase
            nc.vector.tensor_tensor(
                out=sel3,
                in0=t_rel[:, r0:r0 + Wg, None].to_broadcast([P, Wg, S]),
                in1=t_iota[:, None, :].to_broadcast([P, Wg, S]),
                op=mybir.AluOpType.is_equal)
            gtiles = {}
            for b, (sbase, Nk) in sorted(meta.items()):
                i0 = ty["ioff"][b] + sbase // 16
                t_gb = gpool.tile([P, (Nk // P) * D], bf16, tag="gb")
                nc.gpsimd.dma_gather(
                    out_ap=t_gb[:].rearrange("p (k d) -> p k d", k=Nk // P),
                    in_ap=tab[b * BUCK:(b + 1) * BUCK, :],
                    idxs_ap=t_idx[:, i0:i0 + Nk // 16],
                    num_idxs=Nk, num_idxs_reg=Nk, elem_size=D,
                    single_packet=False)
                gtiles[b] = t_gb
            t_ps = appool.tile([P, GROUP], mybir.dt.float32, space="PSUM",
                               tag="agg")
            for w, (b, k, coff, span) in enumerate(wins):
                nc.tensor.matmul(
                    t_ps[:, coff:coff + span],
                    lhsT=gtiles[b][:, k * D:(k + 1) * D],
                    rhs=t_sel[:, w * S:w * S + span],
                    start=(w == 0), stop=(w == Wg - 1))
            return t_ps, ncols

        def scale_mean(tname, g, t_ps, ncols):
            """m^T = s^T * (1/cnt)/QS broadcast across partitions -> bf16."""
            o = IVOFF[tname] + g * GROUP
            t_iv = spool.tile([P, GROUP], mybir.dt.float32, tag="iv")
            nc.sync.dma_start(t_iv[:, :ncols],
                              d_iv[0:1, o:o + ncols].to_broadcast([P, ncols]))
            t_m = mpool.tile([P, GROUP], mybir.dt.bfloat16, tag=f"m_{tname}")
            nc.vector.tensor_tensor(out=t_m[:, :ncols], in0=t_ps[:, :ncols],
                                    in1=t_iv[:, :ncols],
                                    op=mybir.AluOpType.mult)
            return t_m

        def xT_blocks(row0, g, ncols):
            """Load node-major int8 code rows for this group and PE-
            transpose into a feature-major [P, ncols] bf16 code tile."""
            t_x = spool.tile([P, GROUP], mybir.dt.bfloat16, tag="xg")
            j0 = 0
            while j0 < ncols:
                w = min(P, ncols - j0)
                t_b8 = gpool.tile([P, P], i8, tag="xblk8")
                nc.sync.dma_start(
                    t_b8[:w, :],
                    d_x8[row0 + g * GROUP + j0:row0 + g * GROUP + j0 + w, :])
                t_blk = gpool.tile([P, P], mybir.dt.bfloat16, tag="xblk")
                nc.vector.tensor_copy(out=t_blk[:w, :], in_=t_b8[:w, :])
                ps_t = trpool.tile([P, P], mybir.dt.bfloat16, space="PSUM",
                                   tag="tr")
                nc.tensor.transpose(ps_t[:, :w], t_blk[:w, :], t_id[:w, :w])
                nc.vector.tensor_copy(out=t_x[:, j0:j0 + w], in_=ps_t[:, :w])
                j0 += w
            return t_x

        def emit_node_major(t_o, dst_dram, g, ncols):
            """PE-transpose feature-major output back to node-major rows."""
            j0 = 0
            while j0 < ncols:
                w = min(P, ncols - j0)
                ps_t = trpool.tile([P, P], mybir.dt.bfloat16, space="PSUM",
                                   tag="tr")
                nc.tensor.transpose(ps_t[:w, :], t_o[:, j0:j0 + w], t_id[:])
                t_nm = gpool.tile([P, P], mybir.dt.bfloat16, tag="nm")
                nc.vector.tensor_copy(out=t_nm[:w, :], in_=ps_t[:w, :])
                nc.sync.dma_start(
                    dst_dram[g * GROUP + j0:g * GROUP + j0 + w, :],
                    t_nm[:w, :])
                j0 += w

        # ---------------- layer 1, s-dst groups (first: frees tab_s1 early)
        wb_bs = 0
        for g in range(len(types["bs"]["groups"])):
            ps_agg, ncols = aggregate("bs", g, wb_bs, tab_b0)
            wb_bs += len(types["bs"]["groups"][g])
            t_m = scale_mean("bs", g, ps_agg, ncols)
            t_x = xT_blocks(NLB, g, ncols)
            ps2 = s2pool.tile([P, GROUP], mybir.dt.float32, space="PSUM",
                              tag="s2")
            nc.tensor.matmul(ps2[:, :ncols], lhsT=wm["Wlbs0"],
                             rhs=t_m[:, :ncols], start=True, stop=False)
            nc.tensor.matmul(ps2[:, :ncols], lhsT=wm["Wrs0"],
                             rhs=t_x[:, :ncols], start=False, stop=True)
            t_o = spool.tile([P, GROUP], mybir.dt.bfloat16, tag="ob")
            nc.scalar.activation(out=t_o[:, :ncols], in_=ps2[:, :ncols],
                                 func=mybir.ActivationFunctionType.Lrelu,
                                 bias=b_bs0, alpha=0.01)
            emit_node_major(t_o, ns_nm, g, ncols)
        nc.gpsimd.collective_compute(
            "AllGather", mybir.AluOpType.bypass, replica_groups=grp,
            ins=[ns_nm[:].opt()], outs=[tab_s1[:].opt()])

        # ---------------- layer 1, b-dst groups
        wb_bb = 0
        wb_sb = 0
        for g in range(len(types["bb"]["groups"])):
            ps_bb, ncols = aggregate("bb", g, wb_bb, tab_b0)
            wb_bb += len(types["bb"]["groups"][g])
            m_bb = scale_mean("bb", g, ps_bb, ncols)
            has_sb = bool(types["sb"]["groups"][g])
            if has_sb:
                ps_sb, ncols_sb = aggregate("sb", g, wb_sb, tab_s0)
                wb_sb += len(types["sb"]["groups"][g])
                m_sb = scale_mean("sb", g, ps_sb, ncols_sb)
            t_x = xT_blocks(0, g, ncols)
            ps2 = s2pool.tile([P, GROUP], mybir.dt.float32, space="PSUM",
                              tag="s2")
            nc.tensor.matmul(ps2[:, :ncols], lhsT=wm["Wlbb0"],
                             rhs=m_bb[:, :ncols], start=True, stop=False)
            if has_sb:
                nc.tensor.matmul(ps2[:, :ncols_sb], lhsT=wm["Wlsb0"],
                                 rhs=m_sb[:, :ncols_sb], start=False,
                                 stop=False)
            nc.tensor.matmul(ps2[:, :ncols], lhsT=wm["Wrb0"],
                             rhs=t_x[:, :ncols], start=False, stop=True)
            t_o = spool.tile([P, GROUP], mybir.dt.bfloat16, tag="ob")
            nc.scalar.activation(out=t_o[:, :ncols], in_=ps2[:, :ncols],
                                 func=mybir.ActivationFunctionType.Lrelu,
                                 bias=b_bb0, alpha=0.01)
            nc.sync.dma_start(d_nbT[:, g * GROUP:g * GROUP + ncols],
                              t_o[:, :ncols])
            emit_node_major(t_o, nb_nm, g, ncols)
        nc.gpsimd.collective_compute(
            "AllGather", mybir.AluOpType.bypass, replica_groups=grp,
            ins=[nb_nm[:].opt()], outs=[tab_b1[:].opt()])

        # ---------------- layer 2, b-dst groups (+ heads)
        # scale_mean reuses the layer-1 (1/cnt)/QS rows; Wlbb1/Wlsb1 were
        # pre-multiplied by QS on the host to compensate.
        wb_bb = 0
        wb_sb = 0
        for g in range(len(types["bb"]["groups"])):
            ps_bb, ncols = aggregate("bb", g, wb_bb, tab_b1)
            wb_bb += len(types["bb"]["groups"][g])
            m_bb = scale_mean("bb", g, ps_bb, ncols)
            has_sb = bool(types["sb"]["groups"][g])
            if has_sb:
                ps_sb, ncols_sb = aggregate("sb", g, wb_sb, tab_s1)
                wb_sb += len(types["sb"]["groups"][g])
                m_sb = scale_mean("sb", g, ps_sb, ncols_sb)
            t_x = spool.tile([P, GROUP], mybir.dt.bfloat16, tag="xg")
            nc.sync.dma_start(t_x[:, :ncols],
                              d_nbT[:, g * GROUP:g * GROUP + ncols])
            ps2 = s2pool.tile([P, GROUP], mybir.dt.float32, space="PSUM",
                              tag="s2")
            nc.tensor.matmul(ps2[:, :ncols], lhsT=wm["Wlbb1"],
                             rhs=m_bb[:, :ncols], start=True, stop=False)
            if has_sb:
                nc.tensor.matmul(ps2[:, :ncols_sb], lhsT=wm["Wlsb1"],
                                 rhs=m_sb[:, :ncols_sb], start=False,
                                 stop=False)
            nc.tensor.matmul(ps2[:, :ncols], lhsT=wm["Wrb1"],
                             rhs=t_x[:, :ncols], start=False, stop=True)
            t_o = spool.tile([P, GROUP], mybir.dt.bfloat16, tag="ob")
            nc.scalar.activation(out=t_o[:, :ncols], in_=ps2[:, :ncols],
                                 func=mybir.ActivationFunctionType.Lrelu,
                                 bias=b_bb1, alpha=0.01)
            ps3 = hpool.tile([8, GROUP], mybir.dt.float32, space="PSUM",
                             tag="hd")
            nc.tensor.matmul(ps3[:, :ncols], lhsT=w_WhT, rhs=t_o[:, :ncols],
                             start=True, stop=True)
            t_y = spool.tile([8, GROUP], mybir.dt.float32, tag="yt")
            nc.vector.tensor_scalar_add(t_y[:, :ncols], ps3[:, :ncols], b_h)
            nc.sync.dma_start(d_yT[:, g * GROUP:g * GROUP + ncols],
                              t_y[:, :ncols])

    nc.compile()
    return nc


LAST_HW_NS = None
LAST_EXEC_S = None
LAST_WARM_S = None


def kernel(x_b, x_s, Wl, bl, Wr, Wh, bh, ei_bb, ei_sb, ei_bs):
    x_b = np.asarray(x_b, np.float32)
    x_s = np.asarray(x_s, np.float32)
    Wl = np.asarray(Wl, np.float32)
    bl = np.asarray(bl, np.float32)
    Wr = np.asarray(Wr, np.float32)
    Wh = np.asarray(Wh, np.float32)
    bh = np.asarray(bh, np.float32)
    ei_bb = np.asarray(ei_bb).astype(np.int64)
    ei_sb = np.asarray(ei_sb).astype(np.int64)
    ei_bs = np.asarray(ei_bs).astype(np.int64)

    # window packing (indices pre-translated into AllGather table rows;
    # identical metadata serves both layers)
    packed = {
        "bb": _prep_type(_tr(ei_bb[0], NLB), ei_bb[1], NB, NLB),
        "sb": _prep_type(_tr(ei_sb[0], NLS), ei_sb[1], NS, NLB),
        "bs": _prep_type(_tr(ei_bs[0], NLB), ei_bs[1], NB, NLS),
    }
    types = {}
    ioff = 0
    roff = 0
    for t in TYPES:
        i16s, rel, _, groups, gb_meta = packed[t]
        offs = []
        for a in i16s:
            offs.append(ioff)
            ioff += a.shape[2]
        types[t] = {"bcols": [a.shape[2] for a in i16s], "ioff": offs,
                    "roff": roff, "Wtot": rel.shape[2],
                    "groups": groups, "gb_meta": gb_meta}
        roff += rel.shape[2]
    nc = _build(types, ioff, roff)

    # weight payload (bf16, with QS folds) + bias columns (f32)
    wmats = [Wl[0, 0], Wl[0, 1], (Wr[0, 0] + Wr[0, 1]) / QS,
             Wl[0, 2], Wr[0, 2] / QS,
             Wl[1, 0] * QS, Wl[1, 1] * QS, Wr[1, 0] + Wr[1, 1]]
    wb_np = np.zeros((P, 8 * D + 8), BF16)
    for i, M in enumerate(wmats):
        wb_np[:, i * D:(i + 1) * D] = M.astype(BF16)
    wb_np[:, 8 * D:8 * D + 8] = Wh.T.astype(BF16)
    bias_np = np.zeros((P, 4), np.float32)
    bias_np[:, 0] = bl[0, 0] + bl[0, 1]
    bias_np[:, 1] = bl[0, 2]
    bias_np[:, 2] = bl[1, 0] + bl[1, 1]
    bias_np[:8, 3] = bh

    def q8(a):
        return np.clip(np.rint(a * QS), -127, 127).astype(np.int8)

    in_maps = []
    for c in range(NCORES):
        im = {
            "x8": np.concatenate([q8(np.ascontiguousarray(x_b[c::NCORES])),
                                  q8(np.ascontiguousarray(x_s[c::NCORES]))],
                                 0),
            "wb": wb_np, "bias": bias_np,
            "iv": np.concatenate(
                [packed["bb"][2][c], packed["sb"][2][c],
                 packed["bs"][2][c]])[None, :] / np.float32(QS),
            "idx": np.concatenate(
                [a[c] for t in TYPES for a in packed[t][0]], 1),
            "rel": np.concatenate([packed[t][1][c] for t in TYPES], 1),
        }
        in_maps.append(im)

    global LAST_HW_NS, LAST_EXEC_S, LAST_WARM_S
    t0 = time.time()
    run_bass_kernel_spmd(nc, in_maps, core_ids=list(range(NCORES)))
    LAST_WARM_S = time.time() - t0

    t0 = time.time()
    res = run_bass_kernel_spmd(nc, in_maps, core_ids=list(range(NCORES)))
    LAST_EXEC_S = (time.time() - t0,)
    LAST_HW_NS = None

    y = np.empty((NB, 8), np.float32)
    for c in range(NCORES):
        y[np.arange(NLB) * NCORES + c] = res.results[c]["yT"].T
    return y


# revision 26
# speedup vs baseline: 154.1293x; 1.1460x over previous
"""HGNN (2-layer hetero GraphSAGE + 8 heads) on 8 trn2 NeuronCores.

Single fused SPMD launch. Nodes are dst-interleaved (core = v % 8,
local = v // 8); each core receives only its node shard (int8 codes,
x ~= code / QS) plus edge window metadata, packed into 6 input arrays
to minimize per-array PJRT overhead. On device:

  1. AllGather the int8 shards into full code tables (core-block row
     order; gather indices pre-translated on host), then widen to bf16
     (codes are exact in bf16; dma_gather wants 256B rows).
  2. Layer 1: per 512-dst-column PSUM group, 128-edge windows (dst-
     sorted, cut on a column grid uniform across all cores so one
     program serves SPMD) are gathered by indirect DMA; a 0/1 selection
     matrix sel[e, j] = (rel[e] == j) from one DVE is_equal feeds PE
     accumulation g.T @ sel -> raw sums s^T; scatter-mean multiplies by
     a DMA-broadcast (1/cnt)/QS row. Dense stage (layer-1 Wr pre-scaled
     by 1/QS) + bias + leaky-relu; outputs are PE-transposed to node-
     major and AllGathered into the layer-2 bf16 tables.
  3. Layer 2 reuses the *same* window metadata against the layer-1
     tables (Wl pre-scaled by QS to reuse the layer-1 1/cnt rows), then
     the 8-head classifier -> yT [8, NLB] f32 per core (only output).

kernel() runs one warm-up launch (hits the persistent jax compilation
cache) then one timed launch; LAST_EXEC_S is the timed launch wall.
"""
import os
import time
import numpy as np

import jax
jax.config.update("jax_compilation_cache_dir",
                  os.path.expanduser("~/.cache/hgnn_jaxcache"))
jax.config.update("jax_persistent_cache_min_entry_size_bytes", -1)
jax.config.update("jax_persistent_cache_min_compile_time_secs", 0.0)

import ml_dtypes
import concourse.bass as bass
import concourse.bacc as bacc
import concourse.mybir as mybir
import concourse.tile as tile
from concourse.bass_utils import run_bass_kernel_spmd

P = 128
D = 128
NCORES = 8
GROUP = 512       # psum columns per accumulation group
S = 128           # max dst-column span per 128-edge window
BUCK = 25000      # src table rows per int16 gather bucket
NB, NS = 100000, 50000
NLB, NLS = NB // NCORES, NS // NCORES   # 12500, 6250
BF16 = ml_dtypes.bfloat16
QS = 26.0         # int8 feature quantization: code = rint(x * QS)
TYPES = ("bb", "sb", "bs")


# ---------------------------------------------------------------- host prep
def _tr(v, nl):
    """Global node id -> row in the core-block AllGather table."""
    return (v % NCORES) * nl + v // NCORES


def _prep_type(src_t, dst, n_tab, n_loc):
    """Shard edges by dst core and pack 128-edge windows on a column grid
    uniform across cores (min-over-cores advance), bucketed by src table
    row so gather indices fit int16.

    src_t: edge source *table rows* (already translated), dst: global dst.
    Returns (idx16: per bucket [NCORES, 16, cols] int16,
             rel   [NCORES, P, Wtot] int8 (-1 pad),
             invc  [NCORES, n_loc] f32,
             groups: per group list of (bucket, k_local, col_off, span),
             gb_meta: per group dict bucket -> (idx slot base, Nk))."""
    nbuck = n_tab // BUCK
    ngroups = -(-n_loc // GROUP)
    core = dst % NCORES
    loc = dst // NCORES
    pcb = [[None] * nbuck for _ in range(NCORES)]
    cumb = [[None] * nbuck for _ in range(NCORES)]
    invc = np.empty((NCORES, n_loc), np.float32)
    for cc in range(NCORES):
        m = core == cc
        s, d = src_t[m], loc[m]
        o = np.argsort(d, kind="stable")
        s, d = s[o], d[o]
        invc[cc] = 1.0 / np.maximum(np.bincount(d, minlength=n_loc), 1)
        for b in range(nbuck):
            mb = (s >= b * BUCK) & (s < (b + 1) * BUCK)
            pcb[cc][b] = (s[mb] - b * BUCK, d[mb])
            cntb = np.bincount(d[mb], minlength=n_loc)
            cumb[cc][b] = np.concatenate([[0], np.cumsum(cntb)])

    groups, gb_meta = [], []
    rel_cols = [[] for _ in range(NCORES)]
    idx_flat = [[[] for _ in range(nbuck)] for _ in range(NCORES)]
    idx_base = [0] * nbuck
    for g in range(ngroups):
        c0, c1 = g * GROUP, min((g + 1) * GROUP, n_loc)
        wins, meta = [], {}
        for b in range(nbuck):
            k_local = 0
            c = c0
            while c < c1:
                span = min(S, c1 - c)
                while span > 1:
                    ok = all(cumb[cc][b][c + span] - cumb[cc][b][c] <= P
                             for cc in range(NCORES))
                    if ok:
                        break
                    span -= 1
                for cc in range(NCORES):
                    s_arr, d_arr = pcb[cc][b]
                    a2, b2 = cumb[cc][b][c], cumb[cc][b][c + span]
                    n = b2 - a2
                    assert n <= P
                    icol = np.zeros(P, np.int16)
                    rcol = np.full(P, -1, np.int8)
                    icol[:n] = s_arr[a2:b2].astype(np.int16)
                    rcol[:n] = (d_arr[a2:b2] - c).astype(np.int8)
                    idx_flat[cc][b].append(icol)
                    rel_cols[cc].append(rcol)
                wins.append((b, k_local, c - c0, span))
                k_local += 1
                c += span
            if k_local:
                meta[b] = (idx_base[b], k_local * P)
                idx_base[b] += k_local * P
        groups.append(wins)
        gb_meta.append(meta)

    idx16 = []
    for b in range(nbuck):
        per_core = []
        for cc in range(NCORES):
            flat = (np.concatenate(idx_flat[cc][b]) if idx_flat[cc][b]
                    else np.zeros(256, np.int16))
            per_core.append(np.ascontiguousarray(flat.reshape(-1, 16).T))
        idx16.append(np.stack(per_core))                 # [NCORES, 16, cols]
    rel = np.stack([np.stack(cs, 1) for cs in rel_cols]).astype(np.int8)
    return idx16, rel, invc, groups, gb_meta


# --------------------------------------------------------------- blob layout
def _layout(totc, totw):
    """Row offsets of each section in the int8 input blob [NRTOT, 128].
    totc is padded to a multiple of 64 idx cols, totw to 128 rel cols."""
    totc_p = -(-totc // 64) * 64
    totw_p = -(-totw // P) * P
    niv_p = -(-(2 * NLB + NLS) // 32) * 32
    off = {}
    off["X0"] = 0
    off["I0"] = NLB + NLS
    off["R0"] = off["I0"] + 16 * 2 * totc_p // P
    off["V0"] = off["R0"] + totw_p
    off["W0"] = off["V0"] + niv_p * 4 // P
    off["B0"] = off["W0"] + P * (WBPAD * 2 // P)
    off["NR"] = off["B0"] + P
    off["totc_p"], off["totw_p"], off["niv_p"] = totc_p, totw_p, niv_p
    return off


WBPAD = 1088      # wb cols padded so each partition stripe is 17 blob rows


# ------------------------------------------------------------- device build
def _build(types, off):
    """types: name -> dict(bcols, ioff (per-bucket col offset into the idx
    section), roff (col offset into the rel section), groups, gb_meta)."""
    nc = bacc.Bacc("TRN2", target_bir_lowering=False, debug=False,
                   num_devices=NCORES)
    f32, bf16 = mybir.dt.float32, mybir.dt.bfloat16
    f16 = mybir.dt.float16
    i16, i8, i32 = mybir.dt.int16, mybir.dt.int8, mybir.dt.int32

    d_blob = nc.dram_tensor("blob", [off["NR"], P], i8, kind="ExternalInput")
    d_yT = nc.dram_tensor("yT", [8, NLB], f16, kind="ExternalOutput")
    IVOFF = {"bb": 0, "sb": NLB, "bs": 2 * NLB}

    # section views:
    # x8 [NLB+NLS, P] i8 node shards; idx [16, totc_p] i16; rel [P, totw_p]
    # i8; iv [1, niv_p] f32; wb [P, WBPAD] bf16 (8 stacked [D, D] mats:
    # Wlbb0 Wlsb0 Wrb0/QS Wlbs0 Wrs0/QS Wlbb1*QS Wlsb1*QS Wrb1, then WhT
    # [D, 8]); bias [P, 32] f32 (cols: bb0 bs0 bb1 bh)
    d_x8 = d_blob
    ap_idx = (d_blob[off["I0"]:off["R0"], :]
              .rearrange("(p q) d -> p (q d)", p=16).bitcast(i16))
    ap_rel = (d_blob[off["R0"]:off["V0"], :]
              .rearrange("(p q) d -> p (q d)", p=P))
    ap_iv = (d_blob[off["V0"]:off["W0"], :]
             .rearrange("(a q) d -> a (q d)", a=1).bitcast(f32))
    ap_wb = (d_blob[off["W0"]:off["B0"], :]
             .rearrange("(p q) d -> p (q d)", p=P).bitcast(bf16))
    ap_bias = d_blob[off["B0"]:off["NR"], :].bitcast(f32)

    from contextlib import ExitStack
    with tile.TileContext(nc) as tc, ExitStack() as ctx:
        wpool = ctx.enter_context(tc.tile_pool(name="w", bufs=1))
        dpool = ctx.enter_context(tc.tile_pool(name="dr", bufs=1, space="DRAM"))
        gpool = ctx.enter_context(tc.tile_pool(name="g", bufs=6))
        selpool = ctx.enter_context(tc.tile_pool(name="sel", bufs=2))
        mpool = ctx.enter_context(tc.tile_pool(name="m", bufs=2))
        spool = ctx.enter_context(tc.tile_pool(name="s", bufs=3))
        appool = ctx.enter_context(tc.tile_pool(name="ap", bufs=3, space="PSUM"))
        s2pool = ctx.enter_context(tc.tile_pool(name="s2", bufs=2, space="PSUM"))
        trpool = ctx.enter_context(tc.tile_pool(name="tr", bufs=2, space="PSUM"))
        hpool = ctx.enter_context(tc.tile_pool(name="h", bufs=1, space="PSUM"))

        # ---- DRAM scratch: bounce shards, gather tables, layer-1 staging
        bounce_b = dpool.tile([NLB, P], i8, tag="bnb")
        bounce_s = dpool.tile([NLS, P], i8, tag="bns")
        tab8_b = dpool.tile([NB, P], i8, tag="t8b")
        tab8_s = dpool.tile([NS, P], i8, tag="t8s")
        tab_b0 = dpool.tile([NB, P], bf16, tag="tb0")
        tab_s0 = dpool.tile([NS, P], bf16, tag="ts0")
        tab_b1 = dpool.tile([NB, P], bf16, tag="tb1")
        tab_s1 = dpool.tile([NS, P], bf16, tag="ts1")
        nb_nm = dpool.tile([NLB, P], bf16, tag="nbm")   # L1 b out, node-major
        ns_nm = dpool.tile([NLS, P], bf16, tag="nsm")
        d_nbT = dpool.tile([P, NLB], bf16, tag="nbt")   # L1 b out, feat-major

        grp = [list(range(NCORES))]
        nc.sync.dma_start(bounce_b[:], d_x8[:NLB, :])
        nc.gpsimd.collective_compute(
            "AllGather", mybir.AluOpType.bypass, replica_groups=grp,
            ins=[bounce_b[:].opt()], outs=[tab8_b[:].opt()])
        nc.sync.dma_start(bounce_s[:], d_x8[NLB:NLB + NLS, :])
        nc.gpsimd.collective_compute(
            "AllGather", mybir.AluOpType.bypass, replica_groups=grp,
            ins=[bounce_s[:].opt()], outs=[tab8_s[:].opt()])

        # widen the int8 code tables to bf16 so dma_gather sees 256B rows
        def cast_range(tab8, tabf, j0, rows):
            if rows >= P:
                b = rows // P
                t8 = gpool.tile([P, b * P], i8, tag="c8")
                tf = gpool.tile([P, b * P], bf16, tag="cf")
                nc.sync.dma_start(
                    t8[:], tab8[j0:j0 + rows, :]
                    .rearrange("(a b) d -> a (b d)", a=P))
                nc.vector.tensor_copy(out=tf[:], in_=t8[:])
                nc.sync.dma_start(
                    tabf[j0:j0 + rows, :]
                    .rearrange("(a b) d -> a (b d)", a=P), tf[:])
            else:
                t8 = gpool.tile([P, P], i8, tag="c8")
                tf = gpool.tile([P, P], bf16, tag="cf")
                nc.sync.dma_start(t8[:rows, :], tab8[j0:j0 + rows, :])
                nc.vector.tensor_copy(out=tf[:rows, :], in_=t8[:rows, :])
                nc.sync.dma_start(tabf[j0:j0 + rows, :], tf[:rows, :])

        for tab8, tabf, n in ((tab8_b, tab_b0, NB), (tab8_s, tab_s0, NS)):
            j0 = 0
            while j0 < n:
                rows = min(16 * P, ((n - j0) // P) * P) or (n - j0)
                cast_range(tab8, tabf, j0, rows)
                j0 += rows

        # ---- constants: weights, iota row, identity
        t_w = wpool.tile([P, WBPAD], bf16, tag="wb")
        nc.sync.dma_start(t_w[:], ap_wb)
        wm = {n: t_w[:, i * D:(i + 1) * D] for i, n in enumerate(
            ["Wlbb0", "Wlsb0", "Wrb0", "Wlbs0", "Wrs0",
             "Wlbb1", "Wlsb1", "Wrb1"])}
        w_WhT = t_w[:, 8 * D:8 * D + 8]
        t_bias = wpool.tile([P, 4], f32, tag="bias")
        nc.sync.dma_start(t_bias[:], ap_bias[:, :4])
        b_bb0, b_bs0, b_bb1 = (t_bias[:, i:i + 1] for i in range(3))
        b_h = t_bias[:8, 3:4]

        t_ii = wpool.tile([P, S], i32, tag="ii")
        nc.gpsimd.iota(t_ii[:], pattern=[[1, S]], base=0, channel_multiplier=0)
        t_iota = wpool.tile([P, S], f32, tag="iota")
        nc.vector.tensor_copy(out=t_iota[:], in_=t_ii[:])
        t_ip = wpool.tile([P, 1], i32, tag="ip")
        nc.gpsimd.iota(t_ip[:], pattern=[[0, 1]], base=0, channel_multiplier=1)
        t_ipf = wpool.tile([P, 1], f32, tag="ipf")
        nc.vector.tensor_copy(out=t_ipf[:], in_=t_ip[:])
        t_id = wpool.tile([P, P], bf16, tag="ident")
        nc.vector.tensor_tensor(out=t_id[:], in0=t_iota[:],
                                in1=t_ipf[:].to_broadcast([P, P]),
                                op=mybir.AluOpType.is_equal)

        # ---- resident idx blob (replicated 16->128 on device) and rel f32
        totc_p, totw_p = off["totc_p"], off["totw_p"]
        t_idx = wpool.tile([P, totc_p], i16, tag="idxb")
        for k in range(8):
            nc.sync.dma_start(t_idx[16 * k:16 * (k + 1), :], ap_idx)
        t_r8 = wpool.tile([P, totw_p], i8, tag="rel8")
        nc.sync.dma_start(t_r8[:], ap_rel)
        t_rel = wpool.tile([P, totw_p], f32, tag="relf")
        nc.vector.tensor_copy(out=t_rel[:], in_=t_r8[:])

        def aggregate(tname, g, wbase, tab):
            """Accumulate one group's scatter-sum into PSUM: returns
            (psum tile [P, GROUP] f32, ncols)."""
            ty = types[tname]
            wins = ty["groups"][g]
            meta = ty["gb_meta"][g]
            Wg = len(wins)
            ncols = max(c + sp for (_, _, c, sp) in wins)
            t_sel = selpool.tile([P, Wg * S], bf16, tag="sel")
            sel3 = t_sel[:].rearrange("p (w s) -> p w s", w=Wg)
            r0 = ty["roff"] + wbase
            nc.vector.tensor_tensor(
                out=sel3,
                in0=t_rel[:, r0:r0 + Wg, None].to_broadcast([P, Wg, S]),
                in1=t_iota[:, None, :].to_broadcast([P, Wg, S]),
                op=mybir.AluOpType.is_equal)
            gtiles = {}
            for b, (sbase, Nk) in sorted(meta.items()):
                i0 = ty["ioff"][b] + sbase // 16
                t_gb = gpool.tile([P, (Nk // P) * D], bf16, tag="gb")
                nc.gpsimd.dma_gather(
                    out_ap=t_gb[:].rearrange("p (k d) -> p k d", k=Nk // P),
                    in_ap=tab[b * BUCK:(b + 1) * BUCK, :],
                    idxs_ap=t_idx[:, i0:i0 + Nk // 16],
                    num_idxs=Nk, num_idxs_reg=Nk, elem_size=D,
                    single_packet=False)
                gtiles[b] = t_gb
            t_ps = appool.tile([P, GROUP], mybir.dt.float32, space="PSUM",
                               tag="agg")
            for w, (b, k, coff, span) in enumerate(wins):
                nc.tensor.matmul(
                    t_ps[:, coff:coff + span],
                    lhsT=gtiles[b][:, k * D:(k + 1) * D],
                    rhs=t_sel[:, w * S:w * S + span],
                    start=(w == 0), stop=(w == Wg - 1))
            return t_ps, ncols

        def scale_mean(tname, g, t_ps, ncols):
            """m^T = s^T * (1/cnt)/QS broadcast across partitions -> bf16."""
            o = IVOFF[tname] + g * GROUP
            t_iv = spool.tile([P, GROUP], mybir.dt.float32, tag="iv")
            nc.sync.dma_start(t_iv[:, :ncols],
                              ap_iv[0:1, o:o + ncols].to_broadcast([P, ncols]))
            t_m = mpool.tile([P, GROUP], mybir.dt.bfloat16, tag=f"m_{tname}")
            nc.vector.tensor_tensor(out=t_m[:, :ncols], in0=t_ps[:, :ncols],
                                    in1=t_iv[:, :ncols],
                                    op=mybir.AluOpType.mult)
            return t_m

        def xT_blocks(row0, g, ncols):
            """Load node-major int8 code rows for this group and PE-
            transpose into a feature-major [P, ncols] bf16 code tile."""
            t_x = spool.tile([P, GROUP], mybir.dt.bfloat16, tag="xg")
            j0 = 0
            while j0 < ncols:
                w = min(P, ncols - j0)
                t_b8 = gpool.tile([P, P], i8, tag="xblk8")
                nc.sync.dma_start(
                    t_b8[:w, :],
                    d_x8[row0 + g * GROUP + j0:row0 + g * GROUP + j0 + w, :])
                t_blk = gpool.tile([P, P], mybir.dt.bfloat16, tag="xblk")
                nc.vector.tensor_copy(out=t_blk[:w, :], in_=t_b8[:w, :])
                ps_t = trpool.tile([P, P], mybir.dt.bfloat16, space="PSUM",
                                   tag="tr")
                nc.tensor.transpose(ps_t[:, :w], t_blk[:w, :], t_id[:w, :w])
                nc.vector.tensor_copy(out=t_x[:, j0:j0 + w], in_=ps_t[:, :w])
                j0 += w
            return t_x

        def emit_node_major(t_o, dst_dram, g, ncols):
            """PE-transpose feature-major output back to node-major rows."""
            j0 = 0
            while j0 < ncols:
                w = min(P, ncols - j0)
                ps_t = trpool.tile([P, P], mybir.dt.bfloat16, space="PSUM",
                                   tag="tr")
                nc.tensor.transpose(ps_t[:w, :], t_o[:, j0:j0 + w], t_id[:])
                t_nm = gpool.tile([P, P], mybir.dt.bfloat16, tag="nm")
                nc.vector.tensor_copy(out=t_nm[:w, :], in_=ps_t[:w, :])
                nc.sync.dma_start(
                    dst_dram[g * GROUP + j0:g * GROUP + j0 + w, :],
                    t_nm[:w, :])
                j0 += w

        # ---------------- layer 1, s-dst groups (first: frees tab_s1 early)
        wb_bs = 0
        for g in range(len(types["bs"]["groups"])):
            ps_agg, ncols = aggregate("bs", g, wb_bs, tab_b0)
            wb_bs += len(types["bs"]["groups"][g])
            t_m = scale_mean("bs", g, ps_agg, ncols)
            t_x = xT_blocks(NLB, g, ncols)
            ps2 = s2pool.tile([P, GROUP], mybir.dt.float32, space="PSUM",
                              tag="s2")
            nc.tensor.matmul(ps2[:, :ncols], lhsT=wm["Wlbs0"],
                             rhs=t_m[:, :ncols], start=True, stop=False)
            nc.tensor.matmul(ps2[:, :ncols], lhsT=wm["Wrs0"],
                             rhs=t_x[:, :ncols], start=False, stop=True)
            t_o = spool.tile([P, GROUP], mybir.dt.bfloat16, tag="ob")
            nc.scalar.activation(out=t_o[:, :ncols], in_=ps2[:, :ncols],
                                 func=mybir.ActivationFunctionType.Lrelu,
                                 bias=b_bs0, alpha=0.01)
            emit_node_major(t_o, ns_nm, g, ncols)
        nc.gpsimd.collective_compute(
            "AllGather", mybir.AluOpType.bypass, replica_groups=grp,
            ins=[ns_nm[:].opt()], outs=[tab_s1[:].opt()])

        # ---------------- layer 1, b-dst groups
        wb_bb = 0
        wb_sb = 0
        for g in range(len(types["bb"]["groups"])):
            ps_bb, ncols = aggregate("bb", g, wb_bb, tab_b0)
            wb_bb += len(types["bb"]["groups"][g])
            m_bb = scale_mean("bb", g, ps_bb, ncols)
            has_sb = bool(types["sb"]["groups"][g])
            if has_sb:
                ps_sb, ncols_sb = aggregate("sb", g, wb_sb, tab_s0)
                wb_sb += len(types["sb"]["groups"][g])
                m_sb = scale_mean("sb", g, ps_sb, ncols_sb)
            t_x = xT_blocks(0, g, ncols)
            ps2 = s2pool.tile([P, GROUP], mybir.dt.float32, space="PSUM",
                              tag="s2")
            nc.tensor.matmul(ps2[:, :ncols], lhsT=wm["Wlbb0"],
                             rhs=m_bb[:, :ncols], start=True, stop=False)
            if has_sb:
                nc.tensor.matmul(ps2[:, :ncols_sb], lhsT=wm["Wlsb0"],
                                 rhs=m_sb[:, :ncols_sb], start=False,
                                 stop=False)
            nc.tensor.matmul(ps2[:, :ncols], lhsT=wm["Wrb0"],
                             rhs=t_x[:, :ncols], start=False, stop=True)
            t_o = spool.tile([P, GROUP], mybir.dt.bfloat16, tag="ob")
            nc.scalar.activation(out=t_o[:, :ncols], in_=ps2[:, :ncols],
                                 func=mybir.ActivationFunctionType.Lrelu,
                                 bias=b_bb0, alpha=0.01)
            nc.sync.dma_start(d_nbT[:, g * GROUP:g * GROUP + ncols],
                              t_o[:, :ncols])
            emit_node_major(t_o, nb_nm, g, ncols)
        nc.gpsimd.collective_compute(
            "AllGather", mybir.AluOpType.bypass, replica_groups=grp,
            ins=[nb_nm[:].opt()], outs=[tab_b1[:].opt()])

        # ---------------- layer 2, b-dst groups (+ heads)
        # scale_mean reuses the layer-1 (1/cnt)/QS rows; Wlbb1/Wlsb1 were
        # pre-multiplied by QS on the host to compensate.
        wb_bb = 0
        wb_sb = 0
        for g in range(len(types["bb"]["groups"])):
            ps_bb, ncols = aggregate("bb", g, wb_bb, tab_b1)
            wb_bb += len(types["bb"]["groups"][g])
            m_bb = scale_mean("bb", g, ps_bb, ncols)
            has_sb = bool(types["sb"]["groups"][g])
            if has_sb:
                ps_sb, ncols_sb = aggregate("sb", g, wb_sb, tab_s1)
                wb_sb += len(types["sb"]["groups"][g])
                m_sb = scale_mean("sb", g, ps_sb, ncols_sb)
            t_x = spool.tile([P, GROUP], mybir.dt.bfloat16, tag="xg")
            nc.sync.dma_start(t_x[:, :ncols],
                              d_nbT[:, g * GROUP:g * GROUP + ncols])
            ps2 = s2pool.tile([P, GROUP], mybir.dt.float32, space="PSUM",
                              tag="s2")
            nc.tensor.matmul(ps2[:, :ncols], lhsT=wm["Wlbb1"],
                             rhs=m_bb[:, :ncols], start=True, stop=False)
            if has_sb:
                nc.tensor.matmul(ps2[:, :ncols_sb], lhsT=wm["Wlsb1"],
                                 rhs=m_sb[:, :ncols_sb], start=False,
                                 stop=False)
            nc.tensor.matmul(ps2[:, :ncols], lhsT=wm["Wrb1"],
                             rhs=t_x[:, :ncols], start=False, stop=True)
            t_o = spool.tile([P, GROUP], mybir.dt.bfloat16, tag="ob")
            nc.scalar.activation(out=t_o[:, :ncols], in_=ps2[:, :ncols],
                                 func=mybir.ActivationFunctionType.Lrelu,
                                 bias=b_bb1, alpha=0.01)
            ps3 = hpool.tile([8, GROUP], mybir.dt.float32, space="PSUM",
                             tag="hd")
            nc.tensor.matmul(ps3[:, :ncols], lhsT=w_WhT, rhs=t_o[:, :ncols],
                             start=True, stop=True)
            t_y = spool.tile([8, GROUP], f16, tag="yt")
            nc.vector.tensor_scalar_add(t_y[:, :ncols], ps3[:, :ncols], b_h)
            nc.sync.dma_start(d_yT[:, g * GROUP:g * GROUP + ncols],
                              t_y[:, :ncols])

    nc.compile()
    return nc


LAST_HW_NS = None
LAST_EXEC_S = None
LAST_WARM_S = None


def kernel(x_b, x_s, Wl, bl, Wr, Wh, bh, ei_bb, ei_sb, ei_bs):
    x_b = np.asarray(x_b, np.float32)
    x_s = np.asarray(x_s, np.float32)
    Wl = np.asarray(Wl, np.float32)
    bl = np.asarray(bl, np.float32)
    Wr = np.asarray(Wr, np.float32)
    Wh = np.asarray(Wh, np.float32)
    bh = np.asarray(bh, np.float32)
    ei_bb = np.asarray(ei_bb).astype(np.int64)
    ei_sb = np.asarray(ei_sb).astype(np.int64)
    ei_bs = np.asarray(ei_bs).astype(np.int64)

    # window packing (indices pre-translated into AllGather table rows;
    # identical metadata serves both layers)
    packed = {
        "bb": _prep_type(_tr(ei_bb[0], NLB), ei_bb[1], NB, NLB),
        "sb": _prep_type(_tr(ei_sb[0], NLS), ei_sb[1], NS, NLB),
        "bs": _prep_type(_tr(ei_bs[0], NLB), ei_bs[1], NB, NLS),
    }
    types = {}
    ioff = 0
    roff = 0
    for t in TYPES:
        i16s, rel, _, groups, gb_meta = packed[t]
        offs = []
        for a in i16s:
            offs.append(ioff)
            ioff += a.shape[2]
        types[t] = {"bcols": [a.shape[2] for a in i16s], "ioff": offs,
                    "roff": roff, "Wtot": rel.shape[2],
                    "groups": groups, "gb_meta": gb_meta}
        roff += rel.shape[2]
    off = _layout(ioff, roff)
    nc = _build(types, off)

    # weight payload (bf16, with QS folds) + bias columns (f32)
    wmats = [Wl[0, 0], Wl[0, 1], (Wr[0, 0] + Wr[0, 1]) / QS,
             Wl[0, 2], Wr[0, 2] / QS,
             Wl[1, 0] * QS, Wl[1, 1] * QS, Wr[1, 0] + Wr[1, 1]]
    wb_np = np.zeros((P, WBPAD), BF16)
    for i, M in enumerate(wmats):
        wb_np[:, i * D:(i + 1) * D] = M.astype(BF16)
    wb_np[:, 8 * D:8 * D + 8] = Wh.T.astype(BF16)
    bias_np = np.zeros((P, 32), np.float32)
    bias_np[:, 0] = bl[0, 0] + bl[0, 1]
    bias_np[:, 1] = bl[0, 2]
    bias_np[:, 2] = bl[1, 0] + bl[1, 1]
    bias_np[:8, 3] = bh

    def q8(a):
        return np.clip(np.rint(a * QS), -127, 127).astype(np.int8)

    in_maps = []
    for c in range(NCORES):
        idx_np = np.concatenate(
            [a[c] for t in TYPES for a in packed[t][0]], 1)
        idx_pad = np.zeros((16, off["totc_p"]), np.int16)
        idx_pad[:, :idx_np.shape[1]] = idx_np
        rel_np = np.concatenate([packed[t][1][c] for t in TYPES], 1)
        rel_pad = np.full((P, off["totw_p"]), -1, np.int8)
        rel_pad[:, :rel_np.shape[1]] = rel_np
        iv_np = np.concatenate(
            [packed["bb"][2][c], packed["sb"][2][c],
             packed["bs"][2][c]]).astype(np.float32) / np.float32(QS)
        iv_pad = np.zeros(off["niv_p"], np.float32)
        iv_pad[:iv_np.shape[0]] = iv_np
        blob = np.concatenate([
            q8(np.ascontiguousarray(x_b[c::NCORES])).reshape(-1),
            q8(np.ascontiguousarray(x_s[c::NCORES])).reshape(-1),
            idx_pad.reshape(-1).view(np.int8),
            rel_pad.reshape(-1),
            iv_pad.view(np.int8),
            wb_np.reshape(-1).view(np.int8),
            bias_np.reshape(-1).view(np.int8),
        ]).reshape(off["NR"], P)
        in_maps.append({"blob": blob})

    global LAST_HW_NS, LAST_EXEC_S, LAST_WARM_S
    t0 = time.time()
    run_bass_kernel_spmd(nc, in_maps, core_ids=list(range(NCORES)))
    LAST_WARM_S = time.time() - t0

    t0 = time.time()
    res = run_bass_kernel_spmd(nc, in_maps, core_ids=list(range(NCORES)))
    LAST_EXEC_S = (time.time() - t0,)
    LAST_HW_NS = None

    y = np.empty((NB, 8), np.float32)
    for c in range(NCORES):
        y[np.arange(NLB) * NCORES + c] = res.results[c]["yT"].T.astype(
            np.float32)
    return y


# revision 27
# speedup vs baseline: 157.2507x; 1.0203x over previous
"""HGNN (2-layer hetero GraphSAGE + 8 heads) on 8 trn2 NeuronCores.

Single fused SPMD launch. Nodes are dst-interleaved (core = v % 8,
local = v // 8); each core receives only its node shard (int8 codes,
x ~= code / QS) plus edge window metadata, packed into 6 input arrays
to minimize per-array PJRT overhead. On device:

  1. AllGather the int8 shards into full code tables (core-block row
     order; gather indices pre-translated on host), then widen to bf16
     (codes are exact in bf16; dma_gather wants 256B rows).
  2. Layer 1: per 512-dst-column PSUM group, 128-edge windows (dst-
     sorted, cut on a column grid uniform across all cores so one
     program serves SPMD) are gathered by indirect DMA; a 0/1 selection
     matrix sel[e, j] = (rel[e] == j) from one DVE is_equal feeds PE
     accumulation g.T @ sel -> raw sums s^T; scatter-mean multiplies by
     a DMA-broadcast (1/cnt)/QS row. Dense stage (layer-1 Wr pre-scaled
     by 1/QS) + bias + leaky-relu; outputs are PE-transposed to node-
     major and AllGathered into the layer-2 bf16 tables.
  3. Layer 2 reuses the *same* window metadata against the layer-1
     tables (Wl pre-scaled by QS to reuse the layer-1 1/cnt rows), then
     the 8-head classifier -> yT [8, NLB] f32 per core (only output).

kernel() runs one warm-up launch (hits the persistent jax compilation
cache) then one timed launch; LAST_EXEC_S is the timed launch wall.
"""
import os
import time
import numpy as np

import jax
jax.config.update("jax_compilation_cache_dir",
                  os.path.expanduser("~/.cache/hgnn_jaxcache"))
jax.config.update("jax_persistent_cache_min_entry_size_bytes", -1)
jax.config.update("jax_persistent_cache_min_compile_time_secs", 0.0)

import ml_dtypes
import concourse.bacc as bacc
import concourse.mybir as mybir
import concourse.tile as tile
from concourse.bass_utils import run_bass_kernel_spmd

P = 128
D = 128
NCORES = 8
GROUP = 512       # psum columns per accumulation group
S = 128           # max dst-column span per 128-edge window
BUCK = 25000      # src table rows per int16 gather bucket
NB, NS = 100000, 50000
NLB, NLS = NB // NCORES, NS // NCORES   # 12500, 6250
BF16 = ml_dtypes.bfloat16
QS = 26.0         # int8 feature quantization: code = rint(x * QS)
TYPES = ("bb", "sb", "bs")


# ---------------------------------------------------------------- host prep
def _tr(v, nl):
    """Global node id -> row in the core-block AllGather table."""
    return (v % NCORES) * nl + v // NCORES


def _prep_type(src_t, dst, n_tab, n_loc):
    """Shard edges by dst core and pack 128-edge windows on a column grid
    uniform across cores (min-over-cores advance), bucketed by src table
    row so gather indices fit int16.

    src_t: edge source *table rows* (already translated), dst: global dst.
    Returns (idx16: per bucket [NCORES, 16, cols] int16,
             rel   [NCORES, P, Wtot] int8 (-1 pad),
             invc  [NCORES, n_loc] f32,
             groups: per group list of (bucket, k_local, col_off, span),
             gb_meta: per group dict bucket -> (idx slot base, Nk))."""
    nbuck = n_tab // BUCK
    ngroups = -(-n_loc // GROUP)
    core = dst % NCORES
    loc = dst // NCORES
    pcb = [[None] * nbuck for _ in range(NCORES)]
    cumb = [[None] * nbuck for _ in range(NCORES)]
    invc = np.empty((NCORES, n_loc), np.float32)
    for cc in range(NCORES):
        m = core == cc
        s, d = src_t[m], loc[m]
        o = np.argsort(d, kind="stable")
        s, d = s[o], d[o]
        invc[cc] = 1.0 / np.maximum(np.bincount(d, minlength=n_loc), 1)
        for b in range(nbuck):
            mb = (s >= b * BUCK) & (s < (b + 1) * BUCK)
            pcb[cc][b] = (s[mb] - b * BUCK, d[mb])
            cntb = np.bincount(d[mb], minlength=n_loc)
            cumb[cc][b] = np.concatenate([[0], np.cumsum(cntb)])

    groups, gb_meta = [], []
    rel_cols = [[] for _ in range(NCORES)]
    idx_flat = [[[] for _ in range(nbuck)] for _ in range(NCORES)]
    idx_base = [0] * nbuck
    for g in range(ngroups):
        c0, c1 = g * GROUP, min((g + 1) * GROUP, n_loc)
        wins, meta = [], {}
        for b in range(nbuck):
            k_local = 0
            c = c0
            while c < c1:
                span = min(S, c1 - c)
                while span > 1:
                    ok = all(cumb[cc][b][c + span] - cumb[cc][b][c] <= P
                             for cc in range(NCORES))
                    if ok:
                        break
                    span -= 1
                for cc in range(NCORES):
                    s_arr, d_arr = pcb[cc][b]
                    a2, b2 = cumb[cc][b][c], cumb[cc][b][c + span]
                    n = b2 - a2
                    assert n <= P
                    icol = np.zeros(P, np.int16)
                    rcol = np.full(P, -1, np.int8)
                    icol[:n] = s_arr[a2:b2].astype(np.int16)
                    rcol[:n] = (d_arr[a2:b2] - c).astype(np.int8)
                    idx_flat[cc][b].append(icol)
                    rel_cols[cc].append(rcol)
                wins.append((b, k_local, c - c0, span))
                k_local += 1
                c += span
            if k_local:
                meta[b] = (idx_base[b], k_local * P)
                idx_base[b] += k_local * P
        groups.append(wins)
        gb_meta.append(meta)

    idx16 = []
    for b in range(nbuck):
        per_core = []
        for cc in range(NCORES):
            flat = (np.concatenate(idx_flat[cc][b]) if idx_flat[cc][b]
                    else np.zeros(256, np.int16))
            per_core.append(np.ascontiguousarray(flat.reshape(-1, 16).T))
        idx16.append(np.stack(per_core))                 # [NCORES, 16, cols]
    rel = np.stack([np.stack(cs, 1) for cs in rel_cols]).astype(np.int8)
    return idx16, rel, invc, groups, gb_meta


# --------------------------------------------------------------- blob layout
def _layout(totc, totw):
    """Row offsets of each section in the int8 input blob [NRTOT, 128].
    totc is padded to a multiple of 64 idx cols, totw to 128 rel cols."""
    totc_p = -(-totc // 64) * 64
    totw_p = -(-totw // P) * P
    niv_p = -(-(2 * NLB + NLS) // 32) * 32
    off = {}
    off["X0"] = 0
    off["I0"] = NLB + NLS
    off["R0"] = off["I0"] + 16 * 2 * totc_p // P
    off["V0"] = off["R0"] + totw_p
    off["W0"] = off["V0"] + niv_p * 4 // P
    off["B0"] = off["W0"] + P * (WBPAD * 2 // P)
    off["NR"] = off["B0"] + P
    off["totc_p"], off["totw_p"], off["niv_p"] = totc_p, totw_p, niv_p
    return off


WBPAD = 1088      # wb cols padded so each partition stripe is 17 blob rows


# ------------------------------------------------------------- device build
def _build(types, off):
    """types: name -> dict(bcols, ioff (per-bucket col offset into the idx
    section), roff (col offset into the rel section), groups, gb_meta)."""
    nc = bacc.Bacc("TRN2", target_bir_lowering=False, debug=False,
                   num_devices=NCORES)
    f32, bf16 = mybir.dt.float32, mybir.dt.bfloat16
    f16 = mybir.dt.float16
    i16, i8, i32 = mybir.dt.int16, mybir.dt.int8, mybir.dt.int32

    d_blob = nc.dram_tensor("blob", [off["NR"], P], i8, kind="ExternalInput")
    d_yT = nc.dram_tensor("yT", [8, NLB], f16, kind="ExternalOutput")
    IVOFF = {"bb": 0, "sb": NLB, "bs": 2 * NLB}

    # section views:
    # x8 [NLB+NLS, P] i8 node shards; idx [16, totc_p] i16; rel [P, totw_p]
    # i8; iv [1, niv_p] f32; wb [P, WBPAD] bf16 (8 stacked [D, D] mats:
    # Wlbb0 Wlsb0 Wrb0/QS Wlbs0 Wrs0/QS Wlbb1*QS Wlsb1*QS Wrb1, then WhT
    # [D, 8]); bias [P, 32] f32 (cols: bb0 bs0 bb1 bh)
    d_x8 = d_blob
    ap_idx = (d_blob[off["I0"]:off["R0"], :]
              .rearrange("(p q) d -> p (q d)", p=16).bitcast(i16))
    ap_rel = (d_blob[off["R0"]:off["V0"], :]
              .rearrange("(p q) d -> p (q d)", p=P))
    ap_iv = (d_blob[off["V0"]:off["W0"], :]
             .rearrange("(a q) d -> a (q d)", a=1).bitcast(f32))
    ap_wb = (d_blob[off["W0"]:off["B0"], :]
             .rearrange("(p q) d -> p (q d)", p=P).bitcast(bf16))
    ap_bias = d_blob[off["B0"]:off["NR"], :].bitcast(f32)

    from contextlib import ExitStack
    with tile.TileContext(nc) as tc, ExitStack() as ctx:
        wpool = ctx.enter_context(tc.tile_pool(name="w", bufs=1))
        dpool = ctx.enter_context(tc.tile_pool(name="dr", bufs=1, space="DRAM"))
        gpool = ctx.enter_context(tc.tile_pool(name="g", bufs=6))
        selpool = ctx.enter_context(tc.tile_pool(name="sel", bufs=2))
        mpool = ctx.enter_context(tc.tile_pool(name="m", bufs=2))
        spool = ctx.enter_context(tc.tile_pool(name="s", bufs=3))
        appool = ctx.enter_context(tc.tile_pool(name="ap", bufs=3, space="PSUM"))
        s2pool = ctx.enter_context(tc.tile_pool(name="s2", bufs=2, space="PSUM"))
        trpool = ctx.enter_context(tc.tile_pool(name="tr", bufs=2, space="PSUM"))
        hpool = ctx.enter_context(tc.tile_pool(name="h", bufs=1, space="PSUM"))

        # ---- DRAM scratch: bounce shards, gather tables, layer-1 staging
        bounce_b = dpool.tile([NLB, P], i8, tag="bnb")
        bounce_s = dpool.tile([NLS, P], i8, tag="bns")
        tab8_b = dpool.tile([NB, P], i8, tag="t8b")
        tab8_s = dpool.tile([NS, P], i8, tag="t8s")
        tab_b0 = dpool.tile([NB, P], bf16, tag="tb0")
        tab_s0 = dpool.tile([NS, P], bf16, tag="ts0")
        tab_b1 = dpool.tile([NB, P], bf16, tag="tb1")
        tab_s1 = dpool.tile([NS, P], bf16, tag="ts1")
        nb_nm = dpool.tile([NLB, P], bf16, tag="nbm")   # L1 b out, node-major
        ns_nm = dpool.tile([NLS, P], bf16, tag="nsm")
        d_nbT = dpool.tile([P, NLB], bf16, tag="nbt")   # L1 b out, feat-major

        grp = [list(range(NCORES))]
        nc.sync.dma_start(bounce_b[:], d_x8[:NLB, :])
        nc.gpsimd.collective_compute(
            "AllGather", mybir.AluOpType.bypass, replica_groups=grp,
            ins=[bounce_b[:].opt()], outs=[tab8_b[:].opt()])
        nc.sync.dma_start(bounce_s[:], d_x8[NLB:NLB + NLS, :])
        nc.gpsimd.collective_compute(
            "AllGather", mybir.AluOpType.bypass, replica_groups=grp,
            ins=[bounce_s[:].opt()], outs=[tab8_s[:].opt()])

        # widen the int8 code tables to bf16 so dma_gather sees 256B rows
        def cast_range(tab8, tabf, j0, rows):
            if rows >= P:
                b = rows // P
                t8 = gpool.tile([P, b * P], i8, tag="c8")
                tf = gpool.tile([P, b * P], bf16, tag="cf")
                nc.sync.dma_start(
                    t8[:], tab8[j0:j0 + rows, :]
                    .rearrange("(a b) d -> a (b d)", a=P))
                nc.vector.tensor_copy(out=tf[:], in_=t8[:])
                nc.sync.dma_start(
                    tabf[j0:j0 + rows, :]
                    .rearrange("(a b) d -> a (b d)", a=P), tf[:])
            else:
                t8 = gpool.tile([P, P], i8, tag="c8")
                tf = gpool.tile([P, P], bf16, tag="cf")
                nc.sync.dma_start(t8[:rows, :], tab8[j0:j0 + rows, :])
                nc.vector.tensor_copy(out=tf[:rows, :], in_=t8[:rows, :])
                nc.sync.dma_start(tabf[j0:j0 + rows, :], tf[:rows, :])

        for tab8, tabf, n in ((tab8_b, tab_b0, NB), (tab8_s, tab_s0, NS)):
            j0 = 0
            while j0 < n:
                rows = min(16 * P, ((n - j0) // P) * P) or (n - j0)
                cast_range(tab8, tabf, j0, rows)
                j0 += rows

        # ---- constants: weights, iota row, identity
        t_w = wpool.tile([P, WBPAD], bf16, tag="wb")
        nc.sync.dma_start(t_w[:], ap_wb)
        wm = {n: t_w[:, i * D:(i + 1) * D] for i, n in enumerate(
            ["Wlbb0", "Wlsb0", "Wrb0", "Wlbs0", "Wrs0",
             "Wlbb1", "Wlsb1", "Wrb1"])}
        w_WhT = t_w[:, 8 * D:8 * D + 8]
        t_bias = wpool.tile([P, 4], f32, tag="bias")
        nc.sync.dma_start(t_bias[:], ap_bias[:, :4])
        b_bb0, b_bs0, b_bb1 = (t_bias[:, i:i + 1] for i in range(3))
        b_h = t_bias[:8, 3:4]

        t_ii = wpool.tile([P, S], i32, tag="ii")
        nc.gpsimd.iota(t_ii[:], pattern=[[1, S]], base=0, channel_multiplier=0)
        t_iota = wpool.tile([P, S], f32, tag="iota")
        nc.vector.tensor_copy(out=t_iota[:], in_=t_ii[:])
        t_ip = wpool.tile([P, 1], i32, tag="ip")
        nc.gpsimd.iota(t_ip[:], pattern=[[0, 1]], base=0, channel_multiplier=1)
        t_ipf = wpool.tile([P, 1], f32, tag="ipf")
        nc.vector.tensor_copy(out=t_ipf[:], in_=t_ip[:])
        t_id = wpool.tile([P, P], bf16, tag="ident")
        nc.vector.tensor_tensor(out=t_id[:], in0=t_iota[:],
                                in1=t_ipf[:].to_broadcast([P, P]),
                                op=mybir.AluOpType.is_equal)

        # ---- resident idx blob (replicated 16->128 on device) and rel f32
        totc_p, totw_p = off["totc_p"], off["totw_p"]
        t_idx = wpool.tile([P, totc_p], i16, tag="idxb")
        for k in range(8):
            nc.sync.dma_start(t_idx[16 * k:16 * (k + 1), :], ap_idx)
        t_r8 = wpool.tile([P, totw_p], i8, tag="rel8")
        nc.sync.dma_start(t_r8[:], ap_rel)
        t_rel = wpool.tile([P, totw_p], f32, tag="relf")
        nc.vector.tensor_copy(out=t_rel[:], in_=t_r8[:])

        def aggregate(tname, g, wbase, tab):
            """Accumulate one group's scatter-sum into PSUM: returns
            (psum tile [P, GROUP] f32, ncols)."""
            ty = types[tname]
            wins = ty["groups"][g]
            meta = ty["gb_meta"][g]
            Wg = len(wins)
            ncols = max(c + sp for (_, _, c, sp) in wins)
            t_sel = selpool.tile([P, Wg * S], bf16, tag="sel")
            sel3 = t_sel[:].rearrange("p (w s) -> p w s", w=Wg)
            r0 = ty["roff"] + wbase
            nc.vector.tensor_tensor(
                out=sel3,
                in0=t_rel[:, r0:r0 + Wg, None].to_broadcast([P, Wg, S]),
                in1=t_iota[:, None, :].to_broadcast([P, Wg, S]),
                op=mybir.AluOpType.is_equal)
            gtiles = {}
            for b, (sbase, Nk) in sorted(meta.items()):
                i0 = ty["ioff"][b] + sbase // 16
                t_gb = gpool.tile([P, (Nk // P) * D], bf16, tag="gb")
                nc.gpsimd.dma_gather(
                    out_ap=t_gb[:].rearrange("p (k d) -> p k d", k=Nk // P),
                    in_ap=tab[b * BUCK:(b + 1) * BUCK, :],
                    idxs_ap=t_idx[:, i0:i0 + Nk // 16],
                    num_idxs=Nk, num_idxs_reg=Nk, elem_size=D,
                    single_packet=False)
                gtiles[b] = t_gb
            t_ps = appool.tile([P, GROUP], mybir.dt.float32, space="PSUM",
                               tag="agg")
            for w, (b, k, coff, span) in enumerate(wins):
                nc.tensor.matmul(
                    t_ps[:, coff:coff + span],
                    lhsT=gtiles[b][:, k * D:(k + 1) * D],
                    rhs=t_sel[:, w * S:w * S + span],
                    start=(w == 0), stop=(w == Wg - 1))
            return t_ps, ncols

        def scale_mean(tname, g, t_ps, ncols):
            """m^T = s^T * (1/cnt)/QS broadcast across partitions -> bf16."""
            o = IVOFF[tname] + g * GROUP
            t_iv = spool.tile([P, GROUP], mybir.dt.float32, tag="iv")
            nc.sync.dma_start(t_iv[:, :ncols],
                              ap_iv[0:1, o:o + ncols].to_broadcast([P, ncols]))
            t_m = mpool.tile([P, GROUP], mybir.dt.bfloat16, tag=f"m_{tname}")
            nc.vector.tensor_tensor(out=t_m[:, :ncols], in0=t_ps[:, :ncols],
                                    in1=t_iv[:, :ncols],
                                    op=mybir.AluOpType.mult)
            return t_m

        def xT_blocks(row0, g, ncols):
            """Load node-major int8 code rows for this group and PE-
            transpose into a feature-major [P, ncols] bf16 code tile."""
            t_x = spool.tile([P, GROUP], mybir.dt.bfloat16, tag="xg")
            j0 = 0
            while j0 < ncols:
                w = min(P, ncols - j0)
                t_b8 = gpool.tile([P, P], i8, tag="xblk8")
                nc.sync.dma_start(
                    t_b8[:w, :],
                    d_x8[row0 + g * GROUP + j0:row0 + g * GROUP + j0 + w, :])
                t_blk = gpool.tile([P, P], mybir.dt.bfloat16, tag="xblk")
                nc.vector.tensor_copy(out=t_blk[:w, :], in_=t_b8[:w, :])
                ps_t = trpool.tile([P, P], mybir.dt.bfloat16, space="PSUM",
                                   tag="tr")
                nc.tensor.transpose(ps_t[:, :w], t_blk[:w, :], t_id[:w, :w])
                nc.vector.tensor_copy(out=t_x[:, j0:j0 + w], in_=ps_t[:, :w])
                j0 += w
            return t_x

        def emit_node_major(t_o, dst_dram, g, ncols):
            """PE-transpose feature-major output back to node-major rows."""
            j0 = 0
            while j0 < ncols:
                w = min(P, ncols - j0)
                ps_t = trpool.tile([P, P], mybir.dt.bfloat16, space="PSUM",
                                   tag="tr")
                nc.tensor.transpose(ps_t[:w, :], t_o[:, j0:j0 + w], t_id[:])
                t_nm = gpool.tile([P, P], mybir.dt.bfloat16, tag="nm")
                nc.vector.tensor_copy(out=t_nm[:w, :], in_=ps_t[:w, :])
                nc.sync.dma_start(
                    dst_dram[g * GROUP + j0:g * GROUP + j0 + w, :],
                    t_nm[:w, :])
                j0 += w

        # ---------------- layer 1, s-dst groups (first: frees tab_s1 early)
        wb_bs = 0
        for g in range(len(types["bs"]["groups"])):
            ps_agg, ncols = aggregate("bs", g, wb_bs, tab_b0)
            wb_bs += len(types["bs"]["groups"][g])
            t_m = scale_mean("bs", g, ps_agg, ncols)
            t_x = xT_blocks(NLB, g, ncols)
            ps2 = s2pool.tile([P, GROUP], mybir.dt.float32, space="PSUM",
                              tag="s2")
            nc.tensor.matmul(ps2[:, :ncols], lhsT=wm["Wlbs0"],
                             rhs=t_m[:, :ncols], start=True, stop=False)
            nc.tensor.matmul(ps2[:, :ncols], lhsT=wm["Wrs0"],
                             rhs=t_x[:, :ncols], start=False, stop=True)
            t_o = spool.tile([P, GROUP], mybir.dt.bfloat16, tag="ob")
            nc.scalar.activation(out=t_o[:, :ncols], in_=ps2[:, :ncols],
                                 func=mybir.ActivationFunctionType.Lrelu,
                                 bias=b_bs0, alpha=0.01)
            emit_node_major(t_o, ns_nm, g, ncols)
        nc.gpsimd.collective_compute(
            "AllGather", mybir.AluOpType.bypass, replica_groups=grp,
            ins=[ns_nm[:].opt()], outs=[tab_s1[:].opt()])

        # ---------------- layer 1, b-dst groups
        wb_bb = 0
        wb_sb = 0
        for g in range(len(types["bb"]["groups"])):
            ps_bb, ncols = aggregate("bb", g, wb_bb, tab_b0)
            wb_bb += len(types["bb"]["groups"][g])
            m_bb = scale_mean("bb", g, ps_bb, ncols)
            has_sb = bool(types["sb"]["groups"][g])
            if has_sb:
                ps_sb, ncols_sb = aggregate("sb", g, wb_sb, tab_s0)
                wb_sb += len(types["sb"]["groups"][g])
                m_sb = scale_mean("sb", g, ps_sb, ncols_sb)
            t_x = xT_blocks(0, g, ncols)
            ps2 = s2pool.tile([P, GROUP], mybir.dt.float32, space="PSUM",
                              tag="s2")
            nc.tensor.matmul(ps2[:, :ncols], lhsT=wm["Wlbb0"],
                             rhs=m_bb[:, :ncols], start=True, stop=False)
            if has_sb:
                nc.tensor.matmul(ps2[:, :ncols_sb], lhsT=wm["Wlsb0"],
                                 rhs=m_sb[:, :ncols_sb], start=False,
                                 stop=False)
            nc.tensor.matmul(ps2[:, :ncols], lhsT=wm["Wrb0"],
                             rhs=t_x[:, :ncols], start=False, stop=True)
            t_o = spool.tile([P, GROUP], mybir.dt.bfloat16, tag="ob")
            nc.scalar.activation(out=t_o[:, :ncols], in_=ps2[:, :ncols],
                                 func=mybir.ActivationFunctionType.Lrelu,
                                 bias=b_bb0, alpha=0.01)
            nc.sync.dma_start(d_nbT[:, g * GROUP:g * GROUP + ncols],
                              t_o[:, :ncols])
            emit_node_major(t_o, nb_nm, g, ncols)
        nc.gpsimd.collective_compute(
            "AllGather", mybir.AluOpType.bypass, replica_groups=grp,
            ins=[nb_nm[:].opt()], outs=[tab_b1[:].opt()])

        # ---------------- layer 2, b-dst groups (+ heads)
        # scale_mean reuses the layer-1 (1/cnt)/QS rows; Wlbb1/Wlsb1 were
        # pre-multiplied by QS on the host to compensate.
        wb_bb = 0
        wb_sb = 0
        for g in range(len(types["bb"]["groups"])):
            ps_bb, ncols = aggregate("bb", g, wb_bb, tab_b1)
            wb_bb += len(types["bb"]["groups"][g])
            m_bb = scale_mean("bb", g, ps_bb, ncols)
            has_sb = bool(types["sb"]["groups"][g])
            if has_sb:
                ps_sb, ncols_sb = aggregate("sb", g, wb_sb, tab_s1)
                wb_sb += len(types["sb"]["groups"][g])
                m_sb = scale_mean("sb", g, ps_sb, ncols_sb)
            t_x = spool.tile([P, GROUP], mybir.dt.bfloat16, tag="xg")
            nc.sync.dma_start(t_x[:, :ncols],
                              d_nbT[:, g * GROUP:g * GROUP + ncols])
            ps2 = s2pool.tile([P, GROUP], mybir.dt.float32, space="PSUM",
                              tag="s2")
            nc.tensor.matmul(ps2[:, :ncols], lhsT=wm["Wlbb1"],
                             rhs=m_bb[:, :ncols], start=True, stop=False)
            if has_sb:
                nc.tensor.matmul(ps2[:, :ncols_sb], lhsT=wm["Wlsb1"],
                                 rhs=m_sb[:, :ncols_sb], start=False,
                                 stop=False)
            nc.tensor.matmul(ps2[:, :ncols], lhsT=wm["Wrb1"],
                             rhs=t_x[:, :ncols], start=False, stop=True)
            t_o = spool.tile([P, GROUP], mybir.dt.bfloat16, tag="ob")
            nc.scalar.activation(out=t_o[:, :ncols], in_=ps2[:, :ncols],
                                 func=mybir.ActivationFunctionType.Lrelu,
                                 bias=b_bb1, alpha=0.01)
            ps3 = hpool.tile([8, GROUP], mybir.dt.float32, space="PSUM",
                             tag="hd")
            nc.tensor.matmul(ps3[:, :ncols], lhsT=w_WhT, rhs=t_o[:, :ncols],
                             start=True, stop=True)
            t_y = spool.tile([8, GROUP], f16, tag="yt")
            nc.vector.tensor_scalar_add(t_y[:, :ncols], ps3[:, :ncols], b_h)
            nc.sync.dma_start(d_yT[:, g * GROUP:g * GROUP + ncols],
                              t_y[:, :ncols])

    nc.compile()
    return nc


LAST_HW_NS = None
LAST_EXEC_S = None
LAST_WARM_S = None


def kernel(x_b, x_s, Wl, bl, Wr, Wh, bh, ei_bb, ei_sb, ei_bs):
    x_b = np.asarray(x_b, np.float32)
    x_s = np.asarray(x_s, np.float32)
    Wl = np.asarray(Wl, np.float32)
    bl = np.asarray(bl, np.float32)
    Wr = np.asarray(Wr, np.float32)
    Wh = np.asarray(Wh, np.float32)
    bh = np.asarray(bh, np.float32)
    ei_bb = np.asarray(ei_bb).astype(np.int64)
    ei_sb = np.asarray(ei_sb).astype(np.int64)
    ei_bs = np.asarray(ei_bs).astype(np.int64)

    # window packing (indices pre-translated into AllGather table rows;
    # identical metadata serves both layers)
    packed = {
        "bb": _prep_type(_tr(ei_bb[0], NLB), ei_bb[1], NB, NLB),
        "sb": _prep_type(_tr(ei_sb[0], NLS), ei_sb[1], NS, NLB),
        "bs": _prep_type(_tr(ei_bs[0], NLB), ei_bs[1], NB, NLS),
    }
    types = {}
    ioff = 0
    roff = 0
    for t in TYPES:
        i16s, rel, _, groups, gb_meta = packed[t]
        offs = []
        for a in i16s:
            offs.append(ioff)
            ioff += a.shape[2]
        types[t] = {"bcols": [a.shape[2] for a in i16s], "ioff": offs,
                    "roff": roff, "Wtot": rel.shape[2],
                    "groups": groups, "gb_meta": gb_meta}
        roff += rel.shape[2]
    off = _layout(ioff, roff)
    nc = _build(types, off)

    # weight payload (bf16, with QS folds) + bias columns (f32)
    wmats = [Wl[0, 0], Wl[0, 1], (Wr[0, 0] + Wr[0, 1]) / QS,
             Wl[0, 2], Wr[0, 2] / QS,
             Wl[1, 0] * QS, Wl[1, 1] * QS, Wr[1, 0] + Wr[1, 1]]
    wb_np = np.zeros((P, WBPAD), BF16)
    for i, M in enumerate(wmats):
        wb_np[:, i * D:(i + 1) * D] = M.astype(BF16)
    wb_np[:, 8 * D:8 * D + 8] = Wh.T.astype(BF16)
    bias_np = np.zeros((P, 32), np.float32)
    bias_np[:, 0] = bl[0, 0] + bl[0, 1]
    bias_np[:, 1] = bl[0, 2]
    bias_np[:, 2] = bl[1, 0] + bl[1, 1]
    bias_np[:8, 3] = bh

    def q8(a):
        return np.clip(np.rint(a * QS), -127, 127).astype(np.int8)

    in_maps = []
    for c in range(NCORES):
        idx_np = np.concatenate(
            [a[c] for t in TYPES for a in packed[t][0]], 1)
        idx_pad = np.zeros((16, off["totc_p"]), np.int16)
        idx_pad[:, :idx_np.shape[1]] = idx_np
        rel_np = np.concatenate([packed[t][1][c] for t in TYPES], 1)
        rel_pad = np.full((P, off["totw_p"]), -1, np.int8)
        rel_pad[:, :rel_np.shape[1]] = rel_np
        iv_np = np.concatenate(
            [packed["bb"][2][c], packed["sb"][2][c],
             packed["bs"][2][c]]).astype(np.float32) / np.float32(QS)
        iv_pad = np.zeros(off["niv_p"], np.float32)
        iv_pad[:iv_np.shape[0]] = iv_np
        blob = np.concatenate([
            q8(np.ascontiguousarray(x_b[c::NCORES])).reshape(-1),
            q8(np.ascontiguousarray(x_s[c::NCORES])).reshape(-1),
            idx_pad.reshape(-1).view(np.int8),
            rel_pad.reshape(-1),
            iv_pad.view(np.int8),
            wb_np.reshape(-1).view(np.int8),
            bias_np.reshape(-1).view(np.int8),
        ]).reshape(off["NR"], P)
        in_maps.append({"blob": blob})

    global LAST_HW_NS, LAST_EXEC_S, LAST_WARM_S
    t0 = time.time()
    run_bass_kernel_spmd(nc, in_maps, core_ids=list(range(NCORES)))
    LAST_WARM_S = time.time() - t0

    t0 = time.time()
    res = run_bass_kernel_spmd(nc, in_maps, core_ids=list(range(NCORES)))
    LAST_EXEC_S = (time.time() - t0,)
    LAST_HW_NS = None

    y = np.empty((NB, 8), np.float32)
    for c in range(NCORES):
        y[np.arange(NLB) * NCORES + c] = res.results[c]["yT"].T.astype(
            np.float32)
    return y


# revision 28
# speedup vs baseline: 162.8249x; 1.0354x over previous
"""HGNN (2-layer hetero GraphSAGE + 8 heads) on 8 trn2 NeuronCores.

Single fused SPMD launch. Nodes are dst-interleaved (core = v % 8,
local = v // 8); each core receives only its node shard (int8 codes,
x ~= code / QS) plus edge window metadata, packed into 6 input arrays
to minimize per-array PJRT overhead. On device:

  1. AllGather the int8 shards into full code tables (core-block row
     order; gather indices pre-translated on host), then widen to bf16
     (codes are exact in bf16; dma_gather wants 256B rows).
  2. Layer 1: per 512-dst-column PSUM group, 128-edge windows (dst-
     sorted, cut on a column grid uniform across all cores so one
     program serves SPMD) are gathered by indirect DMA; a 0/1 selection
     matrix sel[e, j] = (rel[e] == j) from one DVE is_equal feeds PE
     accumulation g.T @ sel -> raw sums s^T; scatter-mean multiplies by
     a DMA-broadcast (1/cnt)/QS row. Dense stage (layer-1 Wr pre-scaled
     by 1/QS) + bias + leaky-relu; outputs are PE-transposed to node-
     major and AllGathered into the layer-2 bf16 tables.
  3. Layer 2 reuses the *same* window metadata against the layer-1
     tables (Wl pre-scaled by QS to reuse the layer-1 1/cnt rows), then
     the 8-head classifier -> yT [8, NLB] f32 per core (only output).

kernel() runs one warm-up launch (hits the persistent jax compilation
cache) then one timed launch; LAST_EXEC_S is the timed launch wall.
"""
import os
import time
import numpy as np

import jax
jax.config.update("jax_compilation_cache_dir",
                  os.path.expanduser("~/.cache/hgnn_jaxcache"))
jax.config.update("jax_persistent_cache_min_entry_size_bytes", -1)
jax.config.update("jax_persistent_cache_min_compile_time_secs", 0.0)

import ml_dtypes
import concourse.bacc as bacc
import concourse.mybir as mybir
import concourse.tile as tile
from concourse.bass_utils import run_bass_kernel_spmd

P = 128
D = 128
NCORES = 8
GROUP = 512       # psum columns per accumulation group
S = 128           # max dst-column span per 128-edge window
BUCK = 25000      # src table rows per int16 gather bucket
NB, NS = 100000, 50000
NLB, NLS = NB // NCORES, NS // NCORES   # 12500, 6250
BF16 = ml_dtypes.bfloat16
QS = 26.0         # int8 feature quantization: code = rint(x * QS)
TYPES = ("bb", "sb", "bs")


# ---------------------------------------------------------------- host prep
def _tr(v, nl):
    """Global node id -> row in the core-block AllGather table."""
    return (v % NCORES) * nl + v // NCORES


def _prep_type(src_t, dst, n_tab, n_loc):
    """Shard edges by dst core and pack 128-edge windows on a column grid
    uniform across cores (min-over-cores advance), bucketed by src table
    row so gather indices fit int16.

    src_t: edge source *table rows* (already translated), dst: global dst.
    Returns (idx16: per bucket [NCORES, 16, cols] int16,
             rel   [NCORES, P, Wtot] int8 (-1 pad),
             invc  [NCORES, n_loc] f32,
             groups: per group list of (bucket, k_local, col_off, span),
             gb_meta: per group dict bucket -> (idx slot base, Nk))."""
    nbuck = n_tab // BUCK
    ngroups = -(-n_loc // GROUP)
    core = dst % NCORES
    loc = dst // NCORES
    pcb = [[None] * nbuck for _ in range(NCORES)]
    cumb = [[None] * nbuck for _ in range(NCORES)]
    invc = np.empty((NCORES, n_loc), np.float32)
    for cc in range(NCORES):
        m = core == cc
        s, d = src_t[m], loc[m]
        o = np.argsort(d, kind="stable")
        s, d = s[o], d[o]
        invc[cc] = 1.0 / np.maximum(np.bincount(d, minlength=n_loc), 1)
        for b in range(nbuck):
            mb = (s >= b * BUCK) & (s < (b + 1) * BUCK)
            pcb[cc][b] = (s[mb] - b * BUCK, d[mb])
            cntb = np.bincount(d[mb], minlength=n_loc)
            cumb[cc][b] = np.concatenate([[0], np.cumsum(cntb)])

    groups, gb_meta = [], []
    rel_cols = [[] for _ in range(NCORES)]
    idx_flat = [[[] for _ in range(nbuck)] for _ in range(NCORES)]
    idx_base = [0] * nbuck
    for g in range(ngroups):
        c0, c1 = g * GROUP, min((g + 1) * GROUP, n_loc)
        wins, meta = [], {}
        for b in range(nbuck):
            k_local = 0
            c = c0
            while c < c1:
                span = min(S, c1 - c)
                while span > 1:
                    ok = all(cumb[cc][b][c + span] - cumb[cc][b][c] <= P
                             for cc in range(NCORES))
                    if ok:
                        break
                    span -= 1
                for cc in range(NCORES):
                    s_arr, d_arr = pcb[cc][b]
                    a2, b2 = cumb[cc][b][c], cumb[cc][b][c + span]
                    n = b2 - a2
                    assert n <= P
                    icol = np.zeros(P, np.int16)
                    rcol = np.full(P, -1, np.int8)
                    icol[:n] = s_arr[a2:b2].astype(np.int16)
                    rcol[:n] = (d_arr[a2:b2] - c).astype(np.int8)
                    idx_flat[cc][b].append(icol)
                    rel_cols[cc].append(rcol)
                wins.append((b, k_local, c - c0, span))
                k_local += 1
                c += span
            if k_local:
                meta[b] = (idx_base[b], k_local * P)
                idx_base[b] += k_local * P
        groups.append(wins)
        gb_meta.append(meta)

    idx16 = []
    for b in range(nbuck):
        per_core = []
        for cc in range(NCORES):
            flat = (np.concatenate(idx_flat[cc][b]) if idx_flat[cc][b]
                    else np.zeros(256, np.int16))
            per_core.append(np.ascontiguousarray(flat.reshape(-1, 16).T))
        idx16.append(np.stack(per_core))                 # [NCORES, 16, cols]
    rel = np.stack([np.stack(cs, 1) for cs in rel_cols]).astype(np.int8)
    return idx16, rel, invc, groups, gb_meta


# --------------------------------------------------------------- blob layout
def _layout(totc, totw):
    """Row offsets of each section in the int8 input blob [NRTOT, 128].
    totc is padded to a multiple of 64 idx cols, totw to 128 rel cols."""
    totc_p = -(-totc // 64) * 64
    totw_p = -(-totw // P) * P
    niv_p = -(-(2 * NLB + NLS) // 32) * 32
    off = {}
    off["X0"] = 0
    off["I0"] = NLB + NLS
    off["R0"] = off["I0"] + 16 * 2 * totc_p // P
    off["V0"] = off["R0"] + totw_p
    off["W0"] = off["V0"] + niv_p * 4 // P
    off["B0"] = off["W0"] + P * (WBPAD * 2 // P)
    off["NR"] = off["B0"] + P
    off["totc_p"], off["totw_p"], off["niv_p"] = totc_p, totw_p, niv_p
    return off


WBPAD = 1088      # wb cols padded so each partition stripe is 17 blob rows


# ------------------------------------------------------------- device build
def _build(types, off):
    """types: name -> dict(bcols, ioff (per-bucket col offset into the idx
    section), roff (col offset into the rel section), groups, gb_meta)."""
    nc = bacc.Bacc("TRN2", target_bir_lowering=False, debug=False,
                   num_devices=NCORES)
    f32, bf16 = mybir.dt.float32, mybir.dt.bfloat16
    f16 = mybir.dt.float16
    i16, i8, i32 = mybir.dt.int16, mybir.dt.int8, mybir.dt.int32

    d_blob = nc.dram_tensor("blob", [off["NR"], P], i8, kind="ExternalInput")
    d_yT = nc.dram_tensor("yT", [8, NLB], f16, kind="ExternalOutput")
    IVOFF = {"bb": 0, "sb": NLB, "bs": 2 * NLB}

    # section views:
    # x8 [NLB+NLS, P] i8 node shards; idx [16, totc_p] i16; rel [P, totw_p]
    # i8; iv [1, niv_p] f32; wb [P, WBPAD] bf16 (8 stacked [D, D] mats:
    # Wlbb0 Wlsb0 Wrb0/QS Wlbs0 Wrs0/QS Wlbb1*QS Wlsb1*QS Wrb1, then WhT
    # [D, 8]); bias [P, 32] f32 (cols: bb0 bs0 bb1 bh)
    d_x8 = d_blob
    ap_idx = (d_blob[off["I0"]:off["R0"], :]
              .rearrange("(p q) d -> p (q d)", p=16).bitcast(i16))
    ap_rel = (d_blob[off["R0"]:off["V0"], :]
              .rearrange("(p q) d -> p (q d)", p=P))
    ap_iv = (d_blob[off["V0"]:off["W0"], :]
             .rearrange("(a q) d -> a (q d)", a=1).bitcast(f32))
    ap_wb = (d_blob[off["W0"]:off["B0"], :]
             .rearrange("(p q) d -> p (q d)", p=P).bitcast(bf16))
    ap_bias = d_blob[off["B0"]:off["NR"], :].bitcast(f32)

    from contextlib import ExitStack
    with tile.TileContext(nc) as tc, ExitStack() as ctx:
        wpool = ctx.enter_context(tc.tile_pool(name="w", bufs=1))
        dpool = ctx.enter_context(tc.tile_pool(name="dr", bufs=1, space="DRAM"))
        gpool = ctx.enter_context(tc.tile_pool(name="g", bufs=6))
        selpool = ctx.enter_context(tc.tile_pool(name="sel", bufs=2))
        mpool = ctx.enter_context(tc.tile_pool(name="m", bufs=2))
        spool = ctx.enter_context(tc.tile_pool(name="s", bufs=3))
        appool = ctx.enter_context(tc.tile_pool(name="ap", bufs=3, space="PSUM"))
        s2pool = ctx.enter_context(tc.tile_pool(name="s2", bufs=2, space="PSUM"))
        trpool = ctx.enter_context(tc.tile_pool(name="tr", bufs=2, space="PSUM"))
        hpool = ctx.enter_context(tc.tile_pool(name="h", bufs=1, space="PSUM"))

        # ---- DRAM scratch: bounce shards, gather tables, layer-1 staging
        bounce_b = dpool.tile([NLB, P], i8, tag="bnb")
        bounce_s = dpool.tile([NLS, P], i8, tag="bns")
        tab8_b = dpool.tile([NB, P], i8, tag="t8b")
        tab8_s = dpool.tile([NS, P], i8, tag="t8s")
        tab_b0 = dpool.tile([NB, P], bf16, tag="tb0")
        tab_s0 = dpool.tile([NS, P], bf16, tag="ts0")
        tab_b1 = dpool.tile([NB, P], bf16, tag="tb1")
        tab_s1 = dpool.tile([NS, P], bf16, tag="ts1")
        nb_nm = dpool.tile([NLB, P], bf16, tag="nbm")   # L1 b out, node-major
        ns_nm = dpool.tile([NLS, P], bf16, tag="nsm")
        d_nbT = dpool.tile([P, NLB], bf16, tag="nbt")   # L1 b out, feat-major

        grp = [list(range(NCORES))]
        nc.sync.dma_start(bounce_b[:], d_x8[:NLB, :])
        nc.gpsimd.collective_compute(
            "AllGather", mybir.AluOpType.bypass, replica_groups=grp,
            ins=[bounce_b[:].opt()], outs=[tab8_b[:].opt()])
        nc.sync.dma_start(bounce_s[:], d_x8[NLB:NLB + NLS, :])
        nc.gpsimd.collective_compute(
            "AllGather", mybir.AluOpType.bypass, replica_groups=grp,
            ins=[bounce_s[:].opt()], outs=[tab8_s[:].opt()])

        # widen the int8 code tables to bf16 so dma_gather sees 256B rows
        def cast_range(tab8, tabf, j0, rows):
            if rows >= P:
                b = rows // P
                t8 = gpool.tile([P, b * P], i8, tag="c8")
                tf = gpool.tile([P, b * P], bf16, tag="cf")
                nc.sync.dma_start(
                    t8[:], tab8[j0:j0 + rows, :]
                    .rearrange("(a b) d -> a (b d)", a=P))
                nc.vector.tensor_copy(out=tf[:], in_=t8[:])
                nc.sync.dma_start(
                    tabf[j0:j0 + rows, :]
                    .rearrange("(a b) d -> a (b d)", a=P), tf[:])
            else:
                t8 = gpool.tile([P, P], i8, tag="c8")
                tf = gpool.tile([P, P], bf16, tag="cf")
                nc.sync.dma_start(t8[:rows, :], tab8[j0:j0 + rows, :])
                nc.vector.tensor_copy(out=tf[:rows, :], in_=t8[:rows, :])
                nc.sync.dma_start(tabf[j0:j0 + rows, :], tf[:rows, :])

        for tab8, tabf, n in ((tab8_b, tab_b0, NB), (tab8_s, tab_s0, NS)):
            j0 = 0
            while j0 < n:
                rows = min(16 * P, ((n - j0) // P) * P) or (n - j0)
                cast_range(tab8, tabf, j0, rows)
                j0 += rows

        # ---- constants: weights, iota row, identity
        t_w = wpool.tile([P, WBPAD], bf16, tag="wb")
        nc.sync.dma_start(t_w[:], ap_wb)
        wm = {n: t_w[:, i * D:(i + 1) * D] for i, n in enumerate(
            ["Wlbb0", "Wlsb0", "Wrb0", "Wlbs0", "Wrs0",
             "Wlbb1", "Wlsb1", "Wrb1"])}
        w_WhT = t_w[:, 8 * D:8 * D + 8]
        t_bias = wpool.tile([P, 4], f32, tag="bias")
        nc.sync.dma_start(t_bias[:], ap_bias[:, :4])
        b_bb0, b_bs0, b_bb1 = (t_bias[:, i:i + 1] for i in range(3))
        b_h = t_bias[:8, 3:4]

        t_ii = wpool.tile([P, S], i32, tag="ii")
        nc.gpsimd.iota(t_ii[:], pattern=[[1, S]], base=0, channel_multiplier=0)
        t_iota = wpool.tile([P, S], f32, tag="iota")
        nc.vector.tensor_copy(out=t_iota[:], in_=t_ii[:])
        t_ip = wpool.tile([P, 1], i32, tag="ip")
        nc.gpsimd.iota(t_ip[:], pattern=[[0, 1]], base=0, channel_multiplier=1)
        t_ipf = wpool.tile([P, 1], f32, tag="ipf")
        nc.vector.tensor_copy(out=t_ipf[:], in_=t_ip[:])
        t_id = wpool.tile([P, P], bf16, tag="ident")
        nc.vector.tensor_tensor(out=t_id[:], in0=t_iota[:],
                                in1=t_ipf[:].to_broadcast([P, P]),
                                op=mybir.AluOpType.is_equal)

        # ---- resident idx blob (replicated 16->128 on device) and rel f32
        totc_p, totw_p = off["totc_p"], off["totw_p"]
        t_idx = wpool.tile([P, totc_p], i16, tag="idxb")
        for k in range(8):
            nc.sync.dma_start(t_idx[16 * k:16 * (k + 1), :], ap_idx)
        t_r8 = wpool.tile([P, totw_p], i8, tag="rel8")
        nc.sync.dma_start(t_r8[:], ap_rel)
        t_rel = wpool.tile([P, totw_p], f32, tag="relf")
        nc.vector.tensor_copy(out=t_rel[:], in_=t_r8[:])

        def aggregate(tname, g, wbase, tab):
            """Accumulate one group's scatter-sum into PSUM: returns
            (psum tile [P, GROUP] f32, ncols)."""
            ty = types[tname]
            wins = ty["groups"][g]
            meta = ty["gb_meta"][g]
            Wg = len(wins)
            ncols = max(c + sp for (_, _, c, sp) in wins)
            t_sel = selpool.tile([P, Wg * S], bf16, tag="sel")
            sel3 = t_sel[:].rearrange("p (w s) -> p w s", w=Wg)
            r0 = ty["roff"] + wbase
            nc.vector.tensor_tensor(
                out=sel3,
                in0=t_rel[:, r0:r0 + Wg, None].to_broadcast([P, Wg, S]),
                in1=t_iota[:, None, :].to_broadcast([P, Wg, S]),
                op=mybir.AluOpType.is_equal)
            gtiles = {}
            for b, (sbase, Nk) in sorted(meta.items()):
                i0 = ty["ioff"][b] + sbase // 16
                t_gb = gpool.tile([P, (Nk // P) * D], bf16, tag="gb")
                nc.gpsimd.dma_gather(
                    out_ap=t_gb[:].rearrange("p (k d) -> p k d", k=Nk // P),
                    in_ap=tab[b * BUCK:(b + 1) * BUCK, :],
                    idxs_ap=t_idx[:, i0:i0 + Nk // 16],
                    num_idxs=Nk, num_idxs_reg=Nk, elem_size=D,
                    single_packet=False)
                gtiles[b] = t_gb
            t_ps = appool.tile([P, GROUP], mybir.dt.float32, space="PSUM",
                               tag="agg")
            for w, (b, k, coff, span) in enumerate(wins):
                nc.tensor.matmul(
                    t_ps[:, coff:coff + span],
                    lhsT=gtiles[b][:, k * D:(k + 1) * D],
                    rhs=t_sel[:, w * S:w * S + span],
                    start=(w == 0), stop=(w == Wg - 1))
            return t_ps, ncols

        def scale_mean(tname, g, t_ps, ncols):
            """m^T = s^T * (1/cnt)/QS broadcast across partitions -> bf16."""
            o = IVOFF[tname] + g * GROUP
            t_iv = spool.tile([P, GROUP], mybir.dt.float32, tag="iv")
            nc.sync.dma_start(t_iv[:, :ncols],
                              ap_iv[0:1, o:o + ncols].to_broadcast([P, ncols]))
            t_m = mpool.tile([P, GROUP], mybir.dt.bfloat16, tag=f"m_{tname}")
            nc.vector.tensor_tensor(out=t_m[:, :ncols], in0=t_ps[:, :ncols],
                                    in1=t_iv[:, :ncols],
                                    op=mybir.AluOpType.mult)
            return t_m

        def xT_blocks(row0, g, ncols):
            """Load node-major int8 code rows for this group and PE-
            transpose into a feature-major [P, ncols] bf16 code tile."""
            t_x = spool.tile([P, GROUP], mybir.dt.bfloat16, tag="xg")
            j0 = 0
            while j0 < ncols:
                w = min(P, ncols - j0)
                t_b8 = gpool.tile([P, P], i8, tag="xblk8")
                nc.sync.dma_start(
                    t_b8[:w, :],
                    d_x8[row0 + g * GROUP + j0:row0 + g * GROUP + j0 + w, :])
                t_blk = gpool.tile([P, P], mybir.dt.bfloat16, tag="xblk")
                nc.vector.tensor_copy(out=t_blk[:w, :], in_=t_b8[:w, :])
                ps_t = trpool.tile([P, P], mybir.dt.bfloat16, space="PSUM",
                                   tag="tr")
                nc.tensor.transpose(ps_t[:, :w], t_blk[:w, :], t_id[:w, :w])
                nc.vector.tensor_copy(out=t_x[:, j0:j0 + w], in_=ps_t[:, :w])
                j0 += w
            return t_x

        def emit_node_major(t_o, dst_dram, g, ncols):
            """PE-transpose feature-major output back to node-major rows."""
            j0 = 0
            while j0 < ncols:
                w = min(P, ncols - j0)
                ps_t = trpool.tile([P, P], mybir.dt.bfloat16, space="PSUM",
                                   tag="tr")
                nc.tensor.transpose(ps_t[:w, :], t_o[:, j0:j0 + w], t_id[:])
                t_nm = gpool.tile([P, P], mybir.dt.bfloat16, tag="nm")
                nc.vector.tensor_copy(out=t_nm[:w, :], in_=ps_t[:w, :])
                nc.sync.dma_start(
                    dst_dram[g * GROUP + j0:g * GROUP + j0 + w, :],
                    t_nm[:w, :])
                j0 += w

        # ---------------- layer 1, s-dst groups (first: frees tab_s1 early)
        wb_bs = 0
        for g in range(len(types["bs"]["groups"])):
            ps_agg, ncols = aggregate("bs", g, wb_bs, tab_b0)
            wb_bs += len(types["bs"]["groups"][g])
            t_m = scale_mean("bs", g, ps_agg, ncols)
            t_x = xT_blocks(NLB, g, ncols)
            ps2 = s2pool.tile([P, GROUP], mybir.dt.float32, space="PSUM",
                              tag="s2")
            nc.tensor.matmul(ps2[:, :ncols], lhsT=wm["Wlbs0"],
                             rhs=t_m[:, :ncols], start=True, stop=False)
            nc.tensor.matmul(ps2[:, :ncols], lhsT=wm["Wrs0"],
                             rhs=t_x[:, :ncols], start=False, stop=True)
            t_o = spool.tile([P, GROUP], mybir.dt.bfloat16, tag="ob")
            nc.scalar.activation(out=t_o[:, :ncols], in_=ps2[:, :ncols],
                                 func=mybir.ActivationFunctionType.Lrelu,
                                 bias=b_bs0, alpha=0.01)
            emit_node_major(t_o, ns_nm, g, ncols)
        nc.gpsimd.collective_compute(
            "AllGather", mybir.AluOpType.bypass, replica_groups=grp,
            ins=[ns_nm[:].opt()], outs=[tab_s1[:].opt()])

        # ---------------- layer 1, b-dst groups
        wb_bb = 0
        wb_sb = 0
        for g in range(len(types["bb"]["groups"])):
            ps_bb, ncols = aggregate("bb", g, wb_bb, tab_b0)
            wb_bb += len(types["bb"]["groups"][g])
            m_bb = scale_mean("bb", g, ps_bb, ncols)
            has_sb = bool(types["sb"]["groups"][g])
            if has_sb:
                ps_sb, ncols_sb = aggregate("sb", g, wb_sb, tab_s0)
                wb_sb += len(types["sb"]["groups"][g])
                m_sb = scale_mean("sb", g, ps_sb, ncols_sb)
            t_x = xT_blocks(0, g, ncols)
            ps2 = s2pool.tile([P, GROUP], mybir.dt.float32, space="PSUM",
                              tag="s2")
            nc.tensor.matmul(ps2[:, :ncols], lhsT=wm["Wlbb0"],
                             rhs=m_bb[:, :ncols], start=True, stop=False)
            if has_sb:
                nc.tensor.matmul(ps2[:, :ncols_sb], lhsT=wm["Wlsb0"],
                                 rhs=m_sb[:, :ncols_sb], start=False,
                                 stop=False)
            nc.tensor.matmul(ps2[:, :ncols], lhsT=wm["Wrb0"],
                             rhs=t_x[:, :ncols], start=False, stop=True)
            t_o = spool.tile([P, GROUP], mybir.dt.bfloat16, tag="ob")
            nc.scalar.activation(out=t_o[:, :ncols], in_=ps2[:, :ncols],
                                 func=mybir.ActivationFunctionType.Lrelu,
                                 bias=b_bb0, alpha=0.01)
            nc.sync.dma_start(d_nbT[:, g * GROUP:g * GROUP + ncols],
                              t_o[:, :ncols])
            emit_node_major(t_o, nb_nm, g, ncols)
        nc.gpsimd.collective_compute(
            "AllGather", mybir.AluOpType.bypass, replica_groups=grp,
            ins=[nb_nm[:].opt()], outs=[tab_b1[:].opt()])

        # ---------------- layer 2, b-dst groups (+ heads)
        # scale_mean reuses the layer-1 (1/cnt)/QS rows; Wlbb1/Wlsb1 were
        # pre-multiplied by QS on the host to compensate.
        wb_bb = 0
        wb_sb = 0
        for g in range(len(types["bb"]["groups"])):
            ps_bb, ncols = aggregate("bb", g, wb_bb, tab_b1)
            wb_bb += len(types["bb"]["groups"][g])
            m_bb = scale_mean("bb", g, ps_bb, ncols)
            has_sb = bool(types["sb"]["groups"][g])
            if has_sb:
                ps_sb, ncols_sb = aggregate("sb", g, wb_sb, tab_s1)
                wb_sb += len(types["sb"]["groups"][g])
                m_sb = scale_mean("sb", g, ps_sb, ncols_sb)
            t_x = spool.tile([P, GROUP], mybir.dt.bfloat16, tag="xg")
            nc.sync.dma_start(t_x[:, :ncols],
                              d_nbT[:, g * GROUP:g * GROUP + ncols])
            ps2 = s2pool.tile([P, GROUP], mybir.dt.float32, space="PSUM",
                              tag="s2")
            nc.tensor.matmul(ps2[:, :ncols], lhsT=wm["Wlbb1"],
                             rhs=m_bb[:, :ncols], start=True, stop=False)
            if has_sb:
                nc.tensor.matmul(ps2[:, :ncols_sb], lhsT=wm["Wlsb1"],
                                 rhs=m_sb[:, :ncols_sb], start=False,
                                 stop=False)
            nc.tensor.matmul(ps2[:, :ncols], lhsT=wm["Wrb1"],
                             rhs=t_x[:, :ncols], start=False, stop=True)
            t_o = spool.tile([P, GROUP], mybir.dt.bfloat16, tag="ob")
            nc.scalar.activation(out=t_o[:, :ncols], in_=ps2[:, :ncols],
                                 func=mybir.ActivationFunctionType.Lrelu,
                                 bias=b_bb1, alpha=0.01)
            ps3 = hpool.tile([8, GROUP], mybir.dt.float32, space="PSUM",
                             tag="hd")
            nc.tensor.matmul(ps3[:, :ncols], lhsT=w_WhT, rhs=t_o[:, :ncols],
                             start=True, stop=True)
            t_y = spool.tile([8, GROUP], f16, tag="yt")
            nc.vector.tensor_scalar_add(t_y[:, :ncols], ps3[:, :ncols], b_h)
            nc.sync.dma_start(d_yT[:, g * GROUP:g * GROUP + ncols],
                              t_y[:, :ncols])

    nc.compile()
    _strip_debug(nc)
    return nc


def _strip_debug(nc):
    """Null per-instruction tracebacks/debug info after compile. They are
    diagnostic-only, dominate the serialized BIR (faster MLIR conversion +
    cache-key hashing per launch), and embed caller file paths / line
    numbers that would make the persistent-compile-cache key depend on the
    call site."""
    try:
        for fn in nc.m.functions:
            for bb in fn.blocks:
                for ins in bb.instructions:
                    ins.debug = None
            for alloc in fn.allocations:
                mls = getattr(alloc, "memorylocations", None) or []
                for ml in mls:
                    if getattr(ml, "ant_debug", None) is not None:
                        ml.ant_debug = None
    except Exception:
        pass


LAST_HW_NS = None
LAST_EXEC_S = None
LAST_WARM_S = None


def kernel(x_b, x_s, Wl, bl, Wr, Wh, bh, ei_bb, ei_sb, ei_bs):
    x_b = np.asarray(x_b, np.float32)
    x_s = np.asarray(x_s, np.float32)
    Wl = np.asarray(Wl, np.float32)
    bl = np.asarray(bl, np.float32)
    Wr = np.asarray(Wr, np.float32)
    Wh = np.asarray(Wh, np.float32)
    bh = np.asarray(bh, np.float32)
    ei_bb = np.asarray(ei_bb).astype(np.int64)
    ei_sb = np.asarray(ei_sb).astype(np.int64)
    ei_bs = np.asarray(ei_bs).astype(np.int64)

    # window packing (indices pre-translated into AllGather table rows;
    # identical metadata serves both layers)
    packed = {
        "bb": _prep_type(_tr(ei_bb[0], NLB), ei_bb[1], NB, NLB),
        "sb": _prep_type(_tr(ei_sb[0], NLS), ei_sb[1], NS, NLB),
        "bs": _prep_type(_tr(ei_bs[0], NLB), ei_bs[1], NB, NLS),
    }
    types = {}
    ioff = 0
    roff = 0
    for t in TYPES:
        i16s, rel, _, groups, gb_meta = packed[t]
        offs = []
        for a in i16s:
            offs.append(ioff)
            ioff += a.shape[2]
        types[t] = {"bcols": [a.shape[2] for a in i16s], "ioff": offs,
                    "roff": roff, "Wtot": rel.shape[2],
                    "groups": groups, "gb_meta": gb_meta}
        roff += rel.shape[2]
    off = _layout(ioff, roff)
    nc = _build(types, off)

    # weight payload (bf16, with QS folds) + bias columns (f32)
    wmats = [Wl[0, 0], Wl[0, 1], (Wr[0, 0] + Wr[0, 1]) / QS,
             Wl[0, 2], Wr[0, 2] / QS,
             Wl[1, 0] * QS, Wl[1, 1] * QS, Wr[1, 0] + Wr[1, 1]]
    wb_np = np.zeros((P, WBPAD), BF16)
    for i, M in enumerate(wmats):
        wb_np[:, i * D:(i + 1) * D] = M.astype(BF16)
    wb_np[:, 8 * D:8 * D + 8] = Wh.T.astype(BF16)
    bias_np = np.zeros((P, 32), np.float32)
    bias_np[:, 0] = bl[0, 0] + bl[0, 1]
    bias_np[:, 1] = bl[0, 2]
    bias_np[:, 2] = bl[1, 0] + bl[1, 1]
    bias_np[:8, 3] = bh

    def q8(a):
        return np.clip(np.rint(a * QS), -127, 127).astype(np.int8)

    in_maps = []
    for c in range(NCORES):
        idx_np = np.concatenate(
            [a[c] for t in TYPES for a in packed[t][0]], 1)
        idx_pad = np.zeros((16, off["totc_p"]), np.int16)
        idx_pad[:, :idx_np.shape[1]] = idx_np
        rel_np = np.concatenate([packed[t][1][c] for t in TYPES], 1)
        rel_pad = np.full((P, off["totw_p"]), -1, np.int8)
        rel_pad[:, :rel_np.shape[1]] = rel_np
        iv_np = np.concatenate(
            [packed["bb"][2][c], packed["sb"][2][c],
             packed["bs"][2][c]]).astype(np.float32) / np.float32(QS)
        iv_pad = np.zeros(off["niv_p"], np.float32)
        iv_pad[:iv_np.shape[0]] = iv_np
        blob = np.concatenate([
            q8(np.ascontiguousarray(x_b[c::NCORES])).reshape(-1),
            q8(np.ascontiguousarray(x_s[c::NCORES])).reshape(-1),
            idx_pad.reshape(-1).view(np.int8),
            rel_pad.reshape(-1),
            iv_pad.view(np.int8),
            wb_np.reshape(-1).view(np.int8),
            bias_np.reshape(-1).view(np.int8),
        ]).reshape(off["NR"], P)
        in_maps.append({"blob": blob})

    global LAST_HW_NS, LAST_EXEC_S, LAST_WARM_S
    t0 = time.time()
    run_bass_kernel_spmd(nc, in_maps, core_ids=list(range(NCORES)))
    LAST_WARM_S = time.time() - t0

    t0 = time.time()
    res = run_bass_kernel_spmd(nc, in_maps, core_ids=list(range(NCORES)))
    LAST_EXEC_S = (time.time() - t0,)
    LAST_HW_NS = None

    y = np.empty((NB, 8), np.float32)
    for c in range(NCORES):
        y[np.arange(NLB) * NCORES + c] = res.results[c]["yT"].T.astype(
            np.float32)
    return y
